# revision 28
# baseline (speedup 1.0000x reference)
"""Trainium2 Bass kernel for causal multi-head attention.

Problem: B=2, S=2048, D=2048, H=16 heads (HD=128), fp32, causal.
Sharding: 8 cores = 2 batches (data parallel) x 4 head-groups (tensor
parallel, 4 heads each). Each core computes Q/K/V projections for its
head slice, causal attention, and a partial out-projection; the host
sums the 4 partials per batch and adds the output bias.

Device layout notes:
  - All operands are bf16 (host pre-casts): every matmul runs at
    1 cycle/row at any moving size, DMA bytes are halved vs fp32, and
    PSUM accumulation stays fp32 so contraction precision is kept.
  - Scores are computed transposed (scores^T[k, q]) so the AV matmul
    uses V in natural [s, d] layout as the stationary operand,
    accumulating ctx^T[d, q] in PSUM over k-tiles.
  - Softmax denominators: exp tiles are accumulated over k-tiles into
    a bf16 SBUF accumulator on the DVE (16-bit DVE fast mode), then a
    single ones-vector matmul per (head, q-chunk) does the 128-way
    partition sum in fp32 PSUM. This removes the per-k-tile denominator
    matmuls from the PE (the bottleneck engine); the bf16 partials cost
    ~1e-3 relative on the denominator since the 128-way sum stays fp32.
  - exp() runs unnormalized (scores are O(6), no max subtraction);
    normalization happens once on ctx^T via a GPSIMD partition
    broadcast of the reciprocal denominators (the idle Pool engine),
    which frees a PSUM bank so the scores pool gets a third buffer
    (deeper PE lookahead over the exp latency).
  - DMA is issued as a few large slab transfers from host-pre-tiled
    DRAM layouts (make_in_maps packs x/w so each transfer is a plain
    2D slice with 4-16KB contiguous per-partition runs), amortizing
    the ~0.6us per-DMA HWDGE issue cost and minimizing descriptors.
"""

import sys

if "/opt/trn_rl_repo" not in sys.path:
    sys.path.insert(0, "/opt/trn_rl_repo")

import numpy as np

import concourse.bacc as bacc
import concourse.mybir as mybir
import concourse.tile as tile
from concourse.bass_utils import run_bass_kernel_spmd
from concourse.masks import make_upper_triangular

B, S, D, H = 2, 2048, 2048, 16
HD = 128                 # head dim
NCORES = 8
HPC = 4                  # heads per core
DC = HPC * HD            # 512: per-core projection width
CT = D // 128            # 16 contraction tiles
QT = S // 512            # 4 query chunks of 512
ST = S // 128            # 16 seq tiles of 128
SCALE = 1.0 / float(np.sqrt(HD))
F32 = mybir.dt.float32
BF16 = mybir.dt.bfloat16
EXP = mybir.ActivationFunctionType.Exp

_BUILT = None


def _build(cfg=None, reps=1):
    cfg = cfg or {}
    STAGE = cfg.get("stage", 3)   # 1: proj only; 2: +attention; 3: full
    SLIM = cfg.get("slimdma", 0)  # >0: truncate input DMAs (timing diag only)
    NONORM = cfg.get("nonorm", 0)  # skip softmax denominator (timing diag only)
    XCB = cfg.get("xcb", 2)    # x chunk slab bufs
    PTB = cfg.get("ptb", 12)   # p^T tile bufs (deep ring: pt lifetime spans
                               # exp -> mask/dacc (DVE queue) -> AV matmul)
    SCB = cfg.get("scb", 3)    # scores psum bufs
    CPB = cfg.get("cpb", 2)    # ctx psum bufs
    PPB = cfg.get("ppb", 2)    # proj psum bufs (per m-tag)
    DAB = cfg.get("dab", 2)    # den accumulator (sbuf) bufs
    OTB = cfg.get("otb", 3)    # out sbuf slab bufs
    nc = bacc.Bacc(trn_type="TRN2", target_bir_lowering=False)
    # inputs are host-pre-tiled so every DMA is a plain 2D slice whose
    # per-partition runs are 4-16KB contiguous (few large descriptors):
    #   xT:  [(n g p), (ct_in_g c)] with x[b,s,d] at [n*512+g*128... see
    #        make_in_maps; slab (n,g) is rows (n*4+g)*128..+128, all cols
    #   wq/wk/wv: [p, (ct, dc)];  wo: [p, (i, d)]
    xT_d = nc.dram_tensor("xT", [QT * 4 * 128, 4 * 512], BF16, kind="ExternalInput")
    wqT_d = nc.dram_tensor("wqT", [128, CT * DC], BF16, kind="ExternalInput")
    wkT_d = nc.dram_tensor("wkT", [128, CT * DC], BF16, kind="ExternalInput")
    wvT_d = nc.dram_tensor("wvT", [128, CT * DC], BF16, kind="ExternalInput")
    woT_d = nc.dram_tensor("woT", [128, HPC * D], BF16, kind="ExternalInput")
    out_d = nc.dram_tensor("out", [S, D], BF16, kind="ExternalOutput")

    with tile.TileContext(nc) as tc:
      for _rep in range(reps):
        _p = f"r{_rep}_"
        with (
            tc.tile_pool(name=_p + "const", bufs=1) as cst,
            tc.tile_pool(name=_p + "persist", bufs=1) as pp,
        ):
            # upper-triangular (incl diagonal) 0/1 mask: allowed = k <= q
            tri_f = cst.tile([128, 128], F32, tag="tri_f", name="tri_f")
            make_upper_triangular(nc, tri_f[:], val=1.0, diag=True)
            tri = cst.tile([128, 128], BF16, tag="tri", name="tri")
            nc.vector.tensor_copy(tri[:], tri_f[:])
            ones_col = cst.tile([128, 1], BF16, tag="ones_col", name="ones_col")
            nc.vector.memset(ones_col[:], 1.0)

            # persistent per-core tensors (partition dim x free dim):
            # qT/kT: per head [HD, S]; v: per s-tile [128, DC]; ctx^T per
            # (head, q-chunk) for fine-grained deps so the out-projection of
            # chunk qt can overlap attention of chunk qt+1
            qTt = [pp.tile([128, S], BF16, tag=f"qT{h}", name=f"qT{h}") for h in range(HPC)]
            kTt = [pp.tile([128, S], BF16, tag=f"kT{h}", name=f"kT{h}") for h in range(HPC)]
            vt = [pp.tile([128, DC], BF16, tag=f"v{s}", name=f"v{s}") for s in range(ST)]
            ctxt = [[pp.tile([128, 512], BF16, tag=f"ctx{h}_{q}", name=f"ctx{h}_{q}")
                     for q in range(QT)] for h in range(HPC)]

            # resident weights, one slab DMA each:
            #   wq/wk/wv: [128, (ct, dc)]  <- [D, DC] DRAM
            #   wo:       [128, (i, d)]    <- [DC, D] DRAM
            wq_sb = pp.tile([128, CT * DC], BF16, tag="wq_sb", name="wq_sb")
            wk_sb = pp.tile([128, CT * DC], BF16, tag="wk_sb", name="wk_sb")
            wv_sb = pp.tile([128, CT * DC], BF16, tag="wv_sb", name="wv_sb")
            wo_sb = pp.tile([128, HPC * D], BF16, tag="wo_sb", name="wo_sb")

            # ---------------- Phase 1: Q/K/V projections ----------------
            with (
                tc.tile_pool(name=_p + "xc", bufs=XCB) as xcp,
                tc.tile_pool(name=_p + "proj_psum", bufs=PPB, space="PSUM") as pps,
            ):
                for n in range(QT):  # s-chunks of 512
                    # x chunk in 4 ct-group sub-slabs so the first matmuls of
                    # chunk 0 can start ~3us in instead of waiting ~25us for
                    # serialized whole-slab DMAs. For n==0 the wq sub-slabs
                    # are interleaved with the x sub-slabs (Q needs both);
                    # wk/wv/wo follow (K/V matmuls run later).
                    xc = xcp.tile([128, CT * 512], BF16, tag="xc", name=f"xc_{n}")
                    # the first chunk's leading sub-slabs are quartered so the
                    # first Q matmuls start earlier
                    gsplit = 4 if n == 0 else 1
                    for g in range(4):
                        r0 = (n * 4 + g) * 128
                        for s in range(gsplit if g == 0 else 1):
                            w = 2048 // (gsplit if g == 0 else 1)
                            we = min(w, SLIM) if SLIM else w
                            nc.sync.dma_start(
                                out=xc[:, g * 2048 + s * w:g * 2048 + s * w + we],
                                in_=xT_d[r0:r0 + 128, s * w:s * w + we],
                            )
                            if n == 0:
                                ww = 4 * DC // (gsplit if g == 0 else 1)
                                wwe = min(ww, SLIM) if SLIM else ww
                                nc.sync.dma_start(
                                    out=wq_sb[:, g * 4 * DC + s * ww:
                                              g * 4 * DC + s * ww + wwe],
                                    in_=wqT_d[:, g * 4 * DC + s * ww:
                                              g * 4 * DC + s * ww + wwe],
                                )
                    if n == 0:
                        # wk split in 4 so K's ct-progressive needs are met
                        # without waiting behind one monolithic transfer;
                        # wv/wo later (V/out-proj matmuls run much later)
                        qw = CT * DC // 4
                        for g4 in range(4):
                            sl = slice(g4 * qw, g4 * qw + (min(qw, SLIM) if SLIM else qw))
                            nc.sync.dma_start(out=wk_sb[:, sl], in_=wkT_d[:, sl])
                        for g2 in range(2):
                            hw_ = CT * DC // 2
                            sl = slice(g2 * hw_, g2 * hw_ + (min(hw_, SLIM) if SLIM else hw_))
                            nc.sync.dma_start(out=wv_sb[:, sl], in_=wvT_d[:, sl])
                        if SLIM:
                            nc.sync.dma_start(out=wo_sb[:, :SLIM], in_=woT_d[:, :SLIM])
                        else:
                            nc.sync.dma_start(out=wo_sb[:], in_=woT_d[:])

                    # Q^T and K^T: out[d-tile(=head) 128, s 512] accum over ct
                    for w_sb, dst in ((wq_sb, qTt), (wk_sb, kTt)):
                        acc = [pps.tile([128, 512], F32, tag=f"acc{m}", name=f"acc_{n}_{m}")
                               for m in range(HPC)]
                        for ct in range(CT):
                            for m in range(HPC):
                                nc.tensor.matmul(
                                    acc[m][:],
                                    (w_sb[:, ct * DC + m * 128:ct * DC + (m + 1) * 128]),
                                    (xc[:, ct * 512:(ct + 1) * 512]),
                                    start=(ct == 0),
                                    stop=(ct == CT - 1),
                                )
                        for m in range(HPC):
                            eng = nc.vector if (m % 2 == 0) else nc.scalar
                            if eng is nc.vector:
                                eng.tensor_copy(dst[m][:, n * 512:(n + 1) * 512], acc[m][:])
                            else:
                                eng.copy(dst[m][:, n * 512:(n + 1) * 512], acc[m][:])

                    # V natural [s-tile 128, d 512]: lhsT = x^T chunk slice
                    accv = [pps.tile([128, 512], F32, tag=f"acc{ss}", name=f"accv_{n}_{ss}")
                            for ss in range(4)]
                    for ct in range(CT):
                        for ss in range(4):
                            nc.tensor.matmul(
                                accv[ss][:],
                                (xc[:, ct * 512 + ss * 128:ct * 512 + (ss + 1) * 128]),
                                (wv_sb[:, ct * DC:(ct + 1) * DC]),
                                start=(ct == 0),
                                stop=(ct == CT - 1),
                            )
                    for ss in range(4):
                        eng = nc.vector if (ss % 2 == 0) else nc.scalar
                        if eng is nc.vector:
                            eng.tensor_copy(vt[n * 4 + ss][:], accv[ss][:])
                        else:
                            eng.copy(vt[n * 4 + ss][:], accv[ss][:])

            if STAGE == 1:
                # timing diag: projections only
                for h in range(HPC):
                    nc.sync.dma_start(out=out_d[h * 128:(h + 1) * 128, :],
                                      in_=qTt[h][:])
                continue

            # ------- Phase 2+3: causal attention with interleaved out-proj ----
            # The PE executes in program order, so the naive per-block order
            # (score MM -> exp on ACT -> AV MM) stalls the PE ~500ns per block
            # waiting for its own exp. Software-pipeline instead: emit score
            # MMs LAG blocks ahead of AV MMs, and pace the PREVIOUS chunk's
            # out-projection matmuls as fillers between AV emissions so the PE
            # always has ready work while ACT catches up. Out-proj PSUM drains
            # go to the otherwise-idle Pool engine (ACT is exp-saturated in
            # this phase).
            with (
                tc.tile_pool(name=_p + "ptp", bufs=PTB) as ptp,
                tc.tile_pool(name=_p + "dap", bufs=DAB) as dap,
                tc.tile_pool(name=_p + "rcp", bufs=2) as rcp,
                tc.tile_pool(name=_p + "rbs", bufs=2) as rbsp,
                tc.tile_pool(name=_p + "osb", bufs=OTB) as osp,
                tc.tile_pool(name=_p + "sc_ps", bufs=SCB, space="PSUM") as scp,
                tc.tile_pool(name=_p + "ctx_ps", bufs=CPB, space="PSUM") as cxp,
                tc.tile_pool(name=_p + "den_ps", bufs=1, space="PSUM") as dnp,
                tc.tile_pool(name=_p + "out_ps", bufs=1, space="PSUM") as ops,
            ):
                LAG = 2  # AV MM trails its score MM by LAG PE blocks

                def make_outproj_thunks(qt, ctx_q):
                    """64 thunks, one PE matmul each; PSUM drains on Pool and
                    the out DMA ride along with the closing matmul of a group."""
                    thunks = []
                    state = {}
                    for r in range(4):
                        q = qt * 4 + r
                        for oc in range(4):
                            for i in range(HPC):
                                def t(qt=qt, q=q, r=r, oc=oc, i=i, ctx_q=ctx_q):
                                    if oc == 0 and i == 0:
                                        state["ot"] = osp.tile(
                                            [128, D], BF16, tag="ot", name=f"ot_{q}")
                                    if i == 0:
                                        state["po"] = ops.tile(
                                            [128, 512], F32, tag=f"po{oc % 2}",
                                            name=f"po_{q}_{oc}")
                                    nc.tensor.matmul(
                                        state["po"][:],
                                        (ctx_q[i][:, r * 128:(r + 1) * 128]),
                                        (wo_sb[:, i * D + oc * 512:i * D + (oc + 1) * 512]),
                                        start=(i == 0),
                                        stop=(i == HPC - 1),
                                    )
                                    if i == HPC - 1:
                                        # Pool can't read PSUM; split drains
                                        # between DVE and ACT
                                        if (r * 4 + oc) % 2 == 0:
                                            nc.vector.tensor_copy(
                                                state["ot"][:, oc * 512:(oc + 1) * 512],
                                                state["po"][:])
                                        else:
                                            nc.scalar.copy(
                                                state["ot"][:, oc * 512:(oc + 1) * 512],
                                                state["po"][:])
                                        if qt == QT - 1 and r == 3:
                                            # kernel tail: ship each quarter as
                                            # soon as it drains
                                            nc.sync.dma_start(
                                                out=out_d[q * 128:(q + 1) * 128,
                                                          oc * 512:(oc + 1) * 512],
                                                in_=state["ot"][:, oc * 512:(oc + 1) * 512])
                                        elif oc == 3:
                                            nc.sync.dma_start(
                                                out=out_d[q * 128:(q + 1) * 128, :],
                                                in_=state["ot"][:])
                                thunks.append(t)
                    return thunks

                pending = []   # out-proj thunks from the previous chunk
                pend_i = 0

                for qt in range(QT):
                    nkt = 4 * qt + 4  # causal: k-tiles 0..4qt+3
                    n_slots = nkt * HPC  # av-emission slots this chunk
                    slot = 0

                    def fillers():
                        # keep pending consumption proportional to progress,
                        # holding back a few thunks to cover the last head's
                        # den chain and the next chunk's out-proj warmup
                        nonlocal pend_i
                        if not pending:
                            return
                        avail = max(0, len(pending) - 6)
                        target = (avail * slot + n_slots - 1) // n_slots
                        while pend_i < min(target, avail):
                            pending[pend_i]()
                            pend_i += 1

                    # NOTE: offloading dacc adds / mask muls to Pool measured
                    # 2.4x WORSE on HW (gpsimd is a software Q7 loop, ~2x DVE
                    # cost per op, and it serialized the phase) — keep Pool to
                    # the 16 partition broadcasts only.
                    ctx_q = []  # per-head normalized ctx^T [128, 512] tiles
                    deferred_den = None
                    for h in range(HPC):
                        cps = cxp.tile([128, 512], F32, tag="cps", name=f"cps_{h}_{qt}")
                        # Denominators: full k-tiles accumulate elementwise in
                        # a bf16 SBUF tile on the DVE (the 128-way k sum folds
                        # into one fp32 PSUM matmul later); the 4 short
                        # DIAGONAL k-tiles instead fold directly on the PE as
                        # ones-vector matmuls accumulated into the den PSUM
                        # tile — the DVE is the binding engine here and the PE
                        # cost of the short tiles is small.
                        dacc = (dap.tile([128, 512], BF16, tag="dacc", name=f"dacc_{h}_{qt}")
                                if qt > 0 else None)
                        pts = {}
                        los = {}
                        den_box = {}

                        def emit_av(kt, cps=cps, h=h, qt=qt, nkt=nkt, pts=pts,
                                    los=los, den_box=den_box):
                            nc.tensor.matmul(
                                cps[:, los[kt]:],
                                (vt[kt][:, h * 128:(h + 1) * 128]),
                                (pts[kt][:, los[kt]:]),
                                start=(kt == 0), stop=(kt == nkt - 1),
                            )
                            if kt >= 4 * qt and not NONORM:
                                # diagonal tile: denominator partial on the PE
                                lo = los[kt]
                                if kt == 4 * qt:
                                    den_box["den"] = dnp.tile(
                                        [1, 512], F32, tag="den", name=f"den_{h}_{qt}")
                                nc.tensor.matmul(
                                    den_box["den"][:, lo:], (ones_col[:]),
                                    (pts[kt][:, lo:]),
                                    start=(kt == 4 * qt),
                                    stop=(qt == 0 and kt == nkt - 1),
                                )

                        for kt in range(nkt):
                            j = kt - 4 * qt
                            # For diagonal blocks only q-cols >= 128j are
                            # unmasked; shrink the matmul N-range to skip the
                            # masked region instead of zero-filling it.
                            lo = 0 if j < 0 else j * 128
                            los[kt] = lo
                            sc = scp.tile([128, 512], F32, tag="sc", name=f"sc_{h}_{qt}_{kt}")
                            nc.tensor.matmul(
                                sc[:, lo:],
                                (kTt[h][:, kt * 128:(kt + 1) * 128]),
                                (qTt[h][:, qt * 512 + lo:(qt + 1) * 512]),
                                start=True,
                                stop=True,
                            )
                            # previous head's denominator matmul slots in here,
                            # one block after its dacc completed (no PE stall)
                            if kt == 1 and deferred_den is not None:
                                deferred_den()
                                deferred_den = None
                            pt = ptp.tile([128, 512], BF16, tag="pt", name=f"pt_{h}_{qt}_{kt}")
                            pts[kt] = pt
                            nc.scalar.activation(
                                pt[:, lo:], sc[:, lo:], EXP, scale=SCALE
                            )
                            if j >= 0:
                                # strictly-diagonal 128x128 sub-block mask
                                nc.vector.tensor_mul(
                                    pt[:, j * 128:(j + 1) * 128],
                                    pt[:, j * 128:(j + 1) * 128],
                                    tri[:],
                                )
                            with nc.allow_low_precision("bf16 den partials; final 128-way sum is fp32 in PSUM"):
                                if NONORM or j >= 0:
                                    pass  # diagonal dens fold on the PE
                                elif kt == 0:
                                    nc.vector.tensor_copy(dacc[:], pt[:])
                                else:
                                    nc.vector.tensor_add(
                                        dacc[:, lo:], dacc[:, lo:], pt[:, lo:])
                            if kt >= LAG:
                                emit_av(kt - LAG)
                                slot += 1
                                fillers()
                        for kt in range(max(0, nkt - LAG), nkt):
                            emit_av(kt)
                            slot += 1
                            fillers()

                        def make_den(h=h, qt=qt, dacc=dacc, cps=cps,
                                     den_box=den_box):
                            def den_thunk():
                                if NONORM:  # timing diag: plain PSUM drain
                                    ctx = ctxt[h][qt]
                                    nc.vector.tensor_copy(ctx[:], cps[:])
                                    ctx_q.append(ctx)
                                    return
                                den = den_box["den"]
                                if dacc is not None:
                                    # fold the full-tile (DVE) partials into
                                    # the diagonal partials already in PSUM
                                    nc.tensor.matmul(
                                        den[:], (ones_col[:]), (dacc[:]),
                                        start=False, stop=True,
                                    )
                                recip = rcp.tile([1, 512], F32, tag="recip", name=f"recip_{h}_{qt}")
                                # ~5x faster than reciprocal(); 18-bit accuracy
                                # is plenty for the softmax denominator and the
                                # den range (>=1, <<1e38) avoids the edge cases
                                nc.vector.reciprocal_approx_fast(recip[:], den[:])
                                # reciprocal broadcast on the Pool engine frees
                                # a PSUM bank (no PE broadcast matmul)
                                rbs = rbsp.tile([128, 512], F32, tag="rbs", name=f"rbs_{h}_{qt}")
                                nc.gpsimd.partition_broadcast(rbs[:], recip[:])
                                ctx = ctxt[h][qt]
                                nc.vector.tensor_mul(ctx[:], cps[:], rbs[:])
                                ctx_q.append(ctx)
                            return den_thunk

                        deferred_den = make_den()
                        if qt == 0:
                            # qt=0 has no dacc matmul and the den PSUM ring is
                            # 1-deep: the next head's first den matmul would
                            # race a deferred reciprocal — emit in place
                            deferred_den()
                            deferred_den = None

                    # last head's den: give the dacc adds a moment by draining
                    # a couple of fillers first
                    if pending and pend_i < len(pending):
                        pending[pend_i]()
                        pend_i += 1
                        if pend_i < len(pending):
                            pending[pend_i]()
                            pend_i += 1
                    if deferred_den is not None:
                        deferred_den()
                        deferred_den = None

                    # flush any remaining fillers, then queue this chunk's
                    # out-projection for interleaving into the next chunk
                    while pend_i < len(pending):
                        pending[pend_i]()
                        pend_i += 1
                    if STAGE >= 3:
                        pending = make_outproj_thunks(qt, ctx_q)
                        pend_i = 0
                    elif qt == QT - 1:
                        # timing diag: attention only; drain ctx tiles
                        for i, ctx in enumerate(ctx_q):
                            nc.sync.dma_start(
                                out=out_d[i * 128:(i + 1) * 128, :512],
                                in_=ctx[:])

                # last chunk's out-projection runs as a straight PE stream
                while pend_i < len(pending):
                    pending[pend_i]()
                    pend_i += 1

    nc.compile()
    return nc


def _get_built():
    global _BUILT
    if _BUILT is None:
        _BUILT = _build()
    return _BUILT


def _bf16(a):
    import ml_dtypes
    return np.ascontiguousarray(a).astype(ml_dtypes.bfloat16)


def make_in_maps(x, wq, wk, wv, wo):
    x = np.asarray(x, dtype=np.float32)
    wq = np.asarray(wq, dtype=np.float32)
    wk = np.asarray(wk, dtype=np.float32)
    wv = np.asarray(wv, dtype=np.float32)
    wo = np.asarray(wo, dtype=np.float32)
    in_maps = []
    for c in range(NCORES):
        b, hg = divmod(c, NCORES // B)
        sl = slice(hg * DC, (hg + 1) * DC)
        # pre-tile for dense-descriptor DMA (see _build dram layout notes)
        xt = x[b].T.reshape(4, 4, 128, 4, 512)          # [g, ct', p, n, c]
        xt = xt.transpose(3, 0, 2, 1, 4).reshape(QT * 4 * 128, 4 * 512)
        wqt = wq[sl, :].T.reshape(CT, 128, DC).transpose(1, 0, 2).reshape(128, CT * DC)
        wkt = wk[sl, :].T.reshape(CT, 128, DC).transpose(1, 0, 2).reshape(128, CT * DC)
        wvt = wv[sl, :].T.reshape(CT, 128, DC).transpose(1, 0, 2).reshape(128, CT * DC)
        wot = wo[:, sl].T.reshape(HPC, 128, D).transpose(1, 0, 2).reshape(128, HPC * D)
        in_maps.append({
            "xT": _bf16(xt),
            "wqT": _bf16(wqt),
            "wkT": _bf16(wkt),
            "wvT": _bf16(wvt),
            "woT": _bf16(wot),
        })
    return in_maps


def combine_outputs(results, bo):
    bo = np.asarray(bo, dtype=np.float32)
    out = np.zeros((B, S, D), dtype=np.float32)
    for c in range(NCORES):
        b = c // (NCORES // B)
        out[b] += np.asarray(results[c]["out"], dtype=np.float32)
    out += bo[None, None, :]
    return out


def kernel(x, wq, wk, wv, wo, bo):
    nc = _get_built()
    in_maps = make_in_maps(x, wq, wk, wv, wo)
    res = run_bass_kernel_spmd(nc, in_maps, core_ids=list(range(NCORES)))
    return combine_outputs(res.results, bo)


if __name__ == "__main__":
    nc = _get_built()
    print("built ok; instructions:", len(nc.inst_map))



# revision 32
# speedup vs baseline: 1.0055x; 1.0055x over previous
"""Trainium2 Bass kernel for causal multi-head attention.

Problem: B=2, S=2048, D=2048, H=16 heads (HD=128), fp32, causal.
Sharding: 8 cores = 2 batches (data parallel) x 4 head-groups (tensor
parallel, 4 heads each). Each core computes Q/K/V projections for its
head slice, causal attention, and a partial out-projection; the host
sums the 4 partials per batch and adds the output bias.

Device layout notes:
  - All operands are bf16 (host pre-casts): every matmul runs at
    1 cycle/row at any moving size, DMA bytes are halved vs fp32, and
    PSUM accumulation stays fp32 so contraction precision is kept.
  - Scores are computed transposed (scores^T[k, q]) so the AV matmul
    uses V in natural [s, d] layout as the stationary operand,
    accumulating ctx^T[d, q] in PSUM over k-tiles.
  - Softmax denominators: exp tiles are accumulated over k-tiles into
    a bf16 SBUF accumulator on the DVE (16-bit DVE fast mode), then a
    single ones-vector matmul per (head, q-chunk) does the 128-way
    partition sum in fp32 PSUM. This removes the per-k-tile denominator
    matmuls from the PE (the bottleneck engine); the bf16 partials cost
    ~1e-3 relative on the denominator since the 128-way sum stays fp32.
  - exp() runs unnormalized (scores are O(6), no max subtraction);
    normalization happens once on ctx^T via a GPSIMD partition
    broadcast of the reciprocal denominators (the idle Pool engine),
    which frees a PSUM bank so the scores pool gets a third buffer
    (deeper PE lookahead over the exp latency).
  - Phase 2/3 is software-pipelined for the in-order PE: score matmuls
    run LAG=2 blocks ahead of the AV matmuls (so the ACT exp latency
    never stalls the PE), the previous chunk's out-projection matmuls
    are paced as fillers between AV emissions, and each head's
    denominator matmul is deferred one block into the next head.
    A deep pt ring (PTB=12) keeps the exp->mask/dacc->AV chain from
    throttling on tile reuse (6 -> 12 measured -35us with the rest).
  - DMA is issued as a few large slab transfers from host-pre-tiled
    DRAM layouts (make_in_maps packs x/w so each transfer is a plain
    2D slice with 4-16KB contiguous per-partition runs), amortizing
    the ~0.6us per-DMA HWDGE issue cost and minimizing descriptors.
  - Measured dead ends (this HW): stationary-weight reuse across
    matmuls (LDWEIGHTS already hidden by the PE reorder window),
    offloading dacc adds / masks to Pool (software Q7 loop, 2.4x
    worse), diagonal den partials as PE ones-matmuls (+21us), fp8
    anywhere in the signal path (e4m3 ~2.4% rms per operand vs the
    2e-2 gate).
"""

import sys

if "/opt/trn_rl_repo" not in sys.path:
    sys.path.insert(0, "/opt/trn_rl_repo")

import numpy as np

import concourse.bacc as bacc
import concourse.mybir as mybir
import concourse.tile as tile
from concourse.bass_utils import run_bass_kernel_spmd
from concourse.masks import make_upper_triangular

B, S, D, H = 2, 2048, 2048, 16
HD = 128                 # head dim
NCORES = 8
HPC = 4                  # heads per core
DC = HPC * HD            # 512: per-core projection width
CT = D // 128            # 16 contraction tiles
QT = S // 512            # 4 query chunks of 512
ST = S // 128            # 16 seq tiles of 128
SCALE = 1.0 / float(np.sqrt(HD))
F32 = mybir.dt.float32
BF16 = mybir.dt.bfloat16
EXP = mybir.ActivationFunctionType.Exp

_BUILT = None


def _build(cfg=None, reps=1):
    cfg = cfg or {}
    STAGE = cfg.get("stage", 3)   # 1: proj only; 2: +attention; 3: full
    SLIM = cfg.get("slimdma", 0)  # >0: truncate input DMAs (timing diag only)
    NONORM = cfg.get("nonorm", 0)  # skip softmax denominator (timing diag only)
    XCB = cfg.get("xcb", 2)    # x chunk slab bufs
    PTB = cfg.get("ptb", 12)   # p^T tile bufs (deep ring: pt lifetime spans
                               # exp -> mask/dacc (DVE queue) -> AV matmul)
    SCB = cfg.get("scb", 3)    # scores psum bufs
    CPB = cfg.get("cpb", 2)    # ctx psum bufs
    PPB = cfg.get("ppb", 2)    # proj psum bufs (per m-tag)
    DAB = cfg.get("dab", 2)    # den accumulator (sbuf) bufs
    OTB = cfg.get("otb", 3)    # out sbuf slab bufs
    nc = bacc.Bacc(trn_type="TRN2", target_bir_lowering=False)
    # inputs are host-pre-tiled so every DMA is a plain 2D slice whose
    # per-partition runs are 4-16KB contiguous (few large descriptors):
    #   xT:  [(n g p), (ct_in_g c)] with x[b,s,d] at [n*512+g*128... see
    #        make_in_maps; slab (n,g) is rows (n*4+g)*128..+128, all cols
    #   wq/wk/wv: [p, (ct, dc)];  wo: [p, (i, d)]
    xT_d = nc.dram_tensor("xT", [QT * 4 * 128, 4 * 512], BF16, kind="ExternalInput")
    wqT_d = nc.dram_tensor("wqT", [128, CT * DC], BF16, kind="ExternalInput")
    wkT_d = nc.dram_tensor("wkT", [128, CT * DC], BF16, kind="ExternalInput")
    wvT_d = nc.dram_tensor("wvT", [128, CT * DC], BF16, kind="ExternalInput")
    woT_d = nc.dram_tensor("woT", [128, HPC * D], BF16, kind="ExternalInput")
    out_d = nc.dram_tensor("out", [S, D], BF16, kind="ExternalOutput")

    with tile.TileContext(nc) as tc:
      for _rep in range(reps):
        _p = f"r{_rep}_"
        with (
            tc.tile_pool(name=_p + "const", bufs=1) as cst,
            tc.tile_pool(name=_p + "persist", bufs=1) as pp,
        ):
            # upper-triangular (incl diagonal) 0/1 mask: allowed = k <= q
            tri_f = cst.tile([128, 128], F32, tag="tri_f", name="tri_f")
            make_upper_triangular(nc, tri_f[:], val=1.0, diag=True)
            tri = cst.tile([128, 128], BF16, tag="tri", name="tri")
            nc.vector.tensor_copy(tri[:], tri_f[:])
            ones_col = cst.tile([128, 1], BF16, tag="ones_col", name="ones_col")
            nc.vector.memset(ones_col[:], 1.0)

            # persistent per-core tensors (partition dim x free dim):
            # qT/kT: per head [HD, S]; v: per s-tile [128, DC]; ctx^T per
            # (head, q-chunk) for fine-grained deps so the out-projection of
            # chunk qt can overlap attention of chunk qt+1
            qTt = [pp.tile([128, S], BF16, tag=f"qT{h}", name=f"qT{h}") for h in range(HPC)]
            kTt = [pp.tile([128, S], BF16, tag=f"kT{h}", name=f"kT{h}") for h in range(HPC)]
            vt = [pp.tile([128, DC], BF16, tag=f"v{s}", name=f"v{s}") for s in range(ST)]
            ctxt = [[pp.tile([128, 512], BF16, tag=f"ctx{h}_{q}", name=f"ctx{h}_{q}")
                     for q in range(QT)] for h in range(HPC)]

            # resident weights, one slab DMA each:
            #   wq/wk/wv: [128, (ct, dc)]  <- [D, DC] DRAM
            #   wo:       [128, (i, d)]    <- [DC, D] DRAM
            wq_sb = pp.tile([128, CT * DC], BF16, tag="wq_sb", name="wq_sb")
            wk_sb = pp.tile([128, CT * DC], BF16, tag="wk_sb", name="wk_sb")
            wv_sb = pp.tile([128, CT * DC], BF16, tag="wv_sb", name="wv_sb")
            wo_sb = pp.tile([128, HPC * D], BF16, tag="wo_sb", name="wo_sb")

            # ---------------- Phase 1: Q/K/V projections ----------------
            with (
                tc.tile_pool(name=_p + "xc", bufs=XCB) as xcp,
                tc.tile_pool(name=_p + "proj_psum", bufs=PPB, space="PSUM") as pps,
            ):
                for n in range(QT):  # s-chunks of 512
                    # x chunk in 4 ct-group sub-slabs so the first matmuls of
                    # chunk 0 can start ~3us in instead of waiting ~25us for
                    # serialized whole-slab DMAs. For n==0 the wq sub-slabs
                    # are interleaved with the x sub-slabs (Q needs both);
                    # wk/wv/wo follow (K/V matmuls run later).
                    xc = xcp.tile([128, CT * 512], BF16, tag="xc", name=f"xc_{n}")
                    # the first chunk's leading sub-slabs are quartered so the
                    # first Q matmuls start earlier
                    gsplit = 4 if n == 0 else 1
                    for g in range(4):
                        r0 = (n * 4 + g) * 128
                        for s in range(gsplit if g == 0 else 1):
                            w = 2048 // (gsplit if g == 0 else 1)
                            we = min(w, SLIM) if SLIM else w
                            nc.sync.dma_start(
                                out=xc[:, g * 2048 + s * w:g * 2048 + s * w + we],
                                in_=xT_d[r0:r0 + 128, s * w:s * w + we],
                            )
                            if n == 0:
                                ww = 4 * DC // (gsplit if g == 0 else 1)
                                wwe = min(ww, SLIM) if SLIM else ww
                                nc.sync.dma_start(
                                    out=wq_sb[:, g * 4 * DC + s * ww:
                                              g * 4 * DC + s * ww + wwe],
                                    in_=wqT_d[:, g * 4 * DC + s * ww:
                                              g * 4 * DC + s * ww + wwe],
                                )
                    if n == 0:
                        # wk split in 4 so K's ct-progressive needs are met
                        # without waiting behind one monolithic transfer;
                        # wv/wo later (V/out-proj matmuls run much later)
                        qw = CT * DC // 4
                        for g4 in range(4):
                            sl = slice(g4 * qw, g4 * qw + (min(qw, SLIM) if SLIM else qw))
                            nc.sync.dma_start(out=wk_sb[:, sl], in_=wkT_d[:, sl])
                        for g2 in range(2):
                            hw_ = CT * DC // 2
                            sl = slice(g2 * hw_, g2 * hw_ + (min(hw_, SLIM) if SLIM else hw_))
                            nc.sync.dma_start(out=wv_sb[:, sl], in_=wvT_d[:, sl])
                        if SLIM:
                            nc.sync.dma_start(out=wo_sb[:, :SLIM], in_=woT_d[:, :SLIM])
                        else:
                            nc.sync.dma_start(out=wo_sb[:], in_=woT_d[:])

                    # Q^T and K^T: out[d-tile(=head) 128, s 512] accum over ct
                    for w_sb, dst in ((wq_sb, qTt), (wk_sb, kTt)):
                        acc = [pps.tile([128, 512], F32, tag=f"acc{m}", name=f"acc_{n}_{m}")
                               for m in range(HPC)]
                        for ct in range(CT):
                            for m in range(HPC):
                                nc.tensor.matmul(
                                    acc[m][:],
                                    (w_sb[:, ct * DC + m * 128:ct * DC + (m + 1) * 128]),
                                    (xc[:, ct * 512:(ct + 1) * 512]),
                                    start=(ct == 0),
                                    stop=(ct == CT - 1),
                                )
                        for m in range(HPC):
                            eng = nc.vector if (m % 2 == 0) else nc.scalar
                            if eng is nc.vector:
                                eng.tensor_copy(dst[m][:, n * 512:(n + 1) * 512], acc[m][:])
                            else:
                                eng.copy(dst[m][:, n * 512:(n + 1) * 512], acc[m][:])

                    # V natural [s-tile 128, d 512]: lhsT = x^T chunk slice
                    accv = [pps.tile([128, 512], F32, tag=f"acc{ss}", name=f"accv_{n}_{ss}")
                            for ss in range(4)]
                    for ct in range(CT):
                        for ss in range(4):
                            nc.tensor.matmul(
                                accv[ss][:],
                                (xc[:, ct * 512 + ss * 128:ct * 512 + (ss + 1) * 128]),
                                (wv_sb[:, ct * DC:(ct + 1) * DC]),
                                start=(ct == 0),
                                stop=(ct == CT - 1),
                            )
                    for ss in range(4):
                        eng = nc.vector if (ss % 2 == 0) else nc.scalar
                        if eng is nc.vector:
                            eng.tensor_copy(vt[n * 4 + ss][:], accv[ss][:])
                        else:
                            eng.copy(vt[n * 4 + ss][:], accv[ss][:])

            if STAGE == 1:
                # timing diag: projections only
                for h in range(HPC):
                    nc.sync.dma_start(out=out_d[h * 128:(h + 1) * 128, :],
                                      in_=qTt[h][:])
                continue

            # ------- Phase 2+3: causal attention with interleaved out-proj ----
            # The PE executes in program order, so the naive per-block order
            # (score MM -> exp on ACT -> AV MM) stalls the PE ~500ns per block
            # waiting for its own exp. Software-pipeline instead: emit score
            # MMs LAG blocks ahead of AV MMs, and pace the PREVIOUS chunk's
            # out-projection matmuls as fillers between AV emissions so the PE
            # always has ready work while ACT catches up. Out-proj PSUM drains
            # go to the otherwise-idle Pool engine (ACT is exp-saturated in
            # this phase).
            with (
                tc.tile_pool(name=_p + "ptp", bufs=PTB) as ptp,
                tc.tile_pool(name=_p + "dap", bufs=DAB) as dap,
                tc.tile_pool(name=_p + "rcp", bufs=2) as rcp,
                tc.tile_pool(name=_p + "rbs", bufs=2) as rbsp,
                tc.tile_pool(name=_p + "osb", bufs=OTB) as osp,
                tc.tile_pool(name=_p + "sc_ps", bufs=SCB, space="PSUM") as scp,
                tc.tile_pool(name=_p + "ctx_ps", bufs=CPB, space="PSUM") as cxp,
                tc.tile_pool(name=_p + "den_ps", bufs=1, space="PSUM") as dnp,
                tc.tile_pool(name=_p + "out_ps", bufs=1, space="PSUM") as ops,
            ):
                LAG = cfg.get("lag", 2)  # AV MM trails its score MM by LAG blocks

                def make_outproj_thunks(qt, ctx_q):
                    """64 thunks, one PE matmul each; PSUM drains on Pool and
                    the out DMA ride along with the closing matmul of a group."""
                    thunks = []
                    state = {}
                    for r in range(4):
                        q = qt * 4 + r
                        for oc in range(4):
                            for i in range(HPC):
                                def t(qt=qt, q=q, r=r, oc=oc, i=i, ctx_q=ctx_q):
                                    if oc == 0 and i == 0:
                                        state["ot"] = osp.tile(
                                            [128, D], BF16, tag="ot", name=f"ot_{q}")
                                    if i == 0:
                                        state["po"] = ops.tile(
                                            [128, 512], F32, tag=f"po{oc % 2}",
                                            name=f"po_{q}_{oc}")
                                    nc.tensor.matmul(
                                        state["po"][:],
                                        (ctx_q[i][:, r * 128:(r + 1) * 128]),
                                        (wo_sb[:, i * D + oc * 512:i * D + (oc + 1) * 512]),
                                        start=(i == 0),
                                        stop=(i == HPC - 1),
                                    )
                                    if i == HPC - 1:
                                        # Pool can't read PSUM; split drains
                                        # between DVE and ACT
                                        if (r * 4 + oc) % 2 == 0:
                                            nc.vector.tensor_copy(
                                                state["ot"][:, oc * 512:(oc + 1) * 512],
                                                state["po"][:])
                                        else:
                                            nc.scalar.copy(
                                                state["ot"][:, oc * 512:(oc + 1) * 512],
                                                state["po"][:])
                                        if qt == QT - 1 and r == 3:
                                            # kernel tail: ship each quarter as
                                            # soon as it drains
                                            nc.sync.dma_start(
                                                out=out_d[q * 128:(q + 1) * 128,
                                                          oc * 512:(oc + 1) * 512],
                                                in_=state["ot"][:, oc * 512:(oc + 1) * 512])
                                        elif oc == 3:
                                            nc.sync.dma_start(
                                                out=out_d[q * 128:(q + 1) * 128, :],
                                                in_=state["ot"][:])
                                thunks.append(t)
                    return thunks

                pending = []   # out-proj thunks from the previous chunk
                pend_i = 0

                for qt in range(QT):
                    nkt = 4 * qt + 4  # causal: k-tiles 0..4qt+3
                    n_slots = nkt * HPC  # av-emission slots this chunk
                    slot = 0

                    def fillers():
                        # keep pending consumption proportional to progress,
                        # holding back a few thunks to cover the last head's
                        # den chain and the next chunk's out-proj warmup
                        nonlocal pend_i
                        if not pending:
                            return
                        avail = max(0, len(pending) - 6)
                        target = (avail * slot + n_slots - 1) // n_slots
                        while pend_i < min(target, avail):
                            pending[pend_i]()
                            pend_i += 1

                    # NOTE: offloading dacc adds / mask muls to Pool measured
                    # 2.4x WORSE on HW (gpsimd is a software Q7 loop, ~2x DVE
                    # cost per op, and it serialized the phase) — keep Pool to
                    # the 16 partition broadcasts only.
                    ctx_q = []  # per-head normalized ctx^T [128, 512] tiles
                    deferred_den = None
                    for h in range(HPC):
                        cps = cxp.tile([128, 512], F32, tag="cps", name=f"cps_{h}_{qt}")
                        # bf16 den accumulator on the DVE; the 128-way k sum
                        # happens once per (h, qt) in fp32 PSUM below, so the
                        # bf16 partial rounding stays ~1e-3 on the denominator.
                        # (Folding the diagonal-tile partials on the PE as
                        # ones-matmuls instead measured +21us: tiny matmuls
                        # disrupt the PE stream more than they relieve DVE.)
                        dacc = dap.tile([128, 512], BF16, tag="dacc", name=f"dacc_{h}_{qt}")
                        pts = {}
                        los = {}

                        def emit_av(kt, cps=cps, h=h, nkt=nkt, pts=pts, los=los):
                            nc.tensor.matmul(
                                cps[:, los[kt]:],
                                (vt[kt][:, h * 128:(h + 1) * 128]),
                                (pts[kt][:, los[kt]:]),
                                start=(kt == 0), stop=(kt == nkt - 1),
                            )

                        for kt in range(nkt):
                            j = kt - 4 * qt
                            # For diagonal blocks only q-cols >= 128j are
                            # unmasked; shrink the matmul N-range to skip the
                            # masked region instead of zero-filling it.
                            lo = 0 if j < 0 else j * 128
                            los[kt] = lo
                            sc = scp.tile([128, 512], F32, tag="sc", name=f"sc_{h}_{qt}_{kt}")
                            nc.tensor.matmul(
                                sc[:, lo:],
                                (kTt[h][:, kt * 128:(kt + 1) * 128]),
                                (qTt[h][:, qt * 512 + lo:(qt + 1) * 512]),
                                start=True,
                                stop=True,
                            )
                            # previous head's denominator matmul slots in here,
                            # one block after its dacc completed (no PE stall)
                            if kt == 1 and deferred_den is not None:
                                deferred_den()
                                deferred_den = None
                            pt = ptp.tile([128, 512], BF16, tag="pt", name=f"pt_{h}_{qt}_{kt}")
                            pts[kt] = pt
                            nc.scalar.activation(
                                pt[:, lo:], sc[:, lo:], EXP, scale=SCALE
                            )
                            if j >= 0:
                                # strictly-diagonal 128x128 sub-block mask
                                nc.vector.tensor_mul(
                                    pt[:, j * 128:(j + 1) * 128],
                                    pt[:, j * 128:(j + 1) * 128],
                                    tri[:],
                                )
                            with nc.allow_low_precision("bf16 den partials; final 128-way sum is fp32 in PSUM"):
                                if NONORM:
                                    pass
                                elif kt == 0:
                                    nc.vector.tensor_copy(dacc[:], pt[:])
                                else:
                                    nc.vector.tensor_add(
                                        dacc[:, lo:], dacc[:, lo:], pt[:, lo:])
                            if kt >= LAG:
                                emit_av(kt - LAG)
                                slot += 1
                                fillers()
                        for kt in range(max(0, nkt - LAG), nkt):
                            emit_av(kt)
                            slot += 1
                            fillers()

                        def make_den(h=h, qt=qt, dacc=dacc, cps=cps):
                            def den_thunk():
                                if NONORM:  # timing diag: plain PSUM drain
                                    ctx = ctxt[h][qt]
                                    nc.vector.tensor_copy(ctx[:], cps[:])
                                    ctx_q.append(ctx)
                                    return
                                den = dnp.tile([1, 512], F32, tag="den", name=f"den_{h}_{qt}")
                                nc.tensor.matmul(
                                    den[:], (ones_col[:]), (dacc[:]),
                                    start=True, stop=True,
                                )
                                recip = rcp.tile([1, 512], F32, tag="recip", name=f"recip_{h}_{qt}")
                                # ~5x faster than reciprocal(); 18-bit accuracy
                                # is plenty for the softmax denominator and the
                                # den range (>=1, <<1e38) avoids the edge cases
                                nc.vector.reciprocal_approx_fast(recip[:], den[:])
                                # reciprocal broadcast on the Pool engine frees
                                # a PSUM bank (no PE broadcast matmul)
                                rbs = rbsp.tile([128, 512], F32, tag="rbs", name=f"rbs_{h}_{qt}")
                                nc.gpsimd.partition_broadcast(rbs[:], recip[:])
                                ctx = ctxt[h][qt]
                                nc.vector.tensor_mul(ctx[:], cps[:], rbs[:])
                                ctx_q.append(ctx)
                            return den_thunk

                        deferred_den = make_den()

                    # last head's den: give the dacc adds a moment by draining
                    # a couple of fillers first
                    if pending and pend_i < len(pending):
                        pending[pend_i]()
                        pend_i += 1
                        if pend_i < len(pending):
                            pending[pend_i]()
                            pend_i += 1
                    if deferred_den is not None:
                        deferred_den()
                        deferred_den = None

                    # flush any remaining fillers, then queue this chunk's
                    # out-projection for interleaving into the next chunk
                    while pend_i < len(pending):
                        pending[pend_i]()
                        pend_i += 1
                    if STAGE >= 3:
                        pending = make_outproj_thunks(qt, ctx_q)
                        pend_i = 0
                    elif qt == QT - 1:
                        # timing diag: attention only; drain ctx tiles
                        for i, ctx in enumerate(ctx_q):
                            nc.sync.dma_start(
                                out=out_d[i * 128:(i + 1) * 128, :512],
                                in_=ctx[:])

                # last chunk's out-projection runs as a straight PE stream
                while pend_i < len(pending):
                    pending[pend_i]()
                    pend_i += 1

    nc.compile()
    return nc


def _get_built():
    global _BUILT
    if _BUILT is None:
        _BUILT = _build()
    return _BUILT


def _bf16(a):
    import ml_dtypes
    return np.ascontiguousarray(a).astype(ml_dtypes.bfloat16)


def make_in_maps(x, wq, wk, wv, wo):
    x = np.asarray(x, dtype=np.float32)
    wq = np.asarray(wq, dtype=np.float32)
    wk = np.asarray(wk, dtype=np.float32)
    wv = np.asarray(wv, dtype=np.float32)
    wo = np.asarray(wo, dtype=np.float32)
    in_maps = []
    for c in range(NCORES):
        b, hg = divmod(c, NCORES // B)
        sl = slice(hg * DC, (hg + 1) * DC)
        # pre-tile for dense-descriptor DMA (see _build dram layout notes)
        xt = x[b].T.reshape(4, 4, 128, 4, 512)          # [g, ct', p, n, c]
        xt = xt.transpose(3, 0, 2, 1, 4).reshape(QT * 4 * 128, 4 * 512)
        wqt = wq[sl, :].T.reshape(CT, 128, DC).transpose(1, 0, 2).reshape(128, CT * DC)
        wkt = wk[sl, :].T.reshape(CT, 128, DC).transpose(1, 0, 2).reshape(128, CT * DC)
        wvt = wv[sl, :].T.reshape(CT, 128, DC).transpose(1, 0, 2).reshape(128, CT * DC)
        wot = wo[:, sl].T.reshape(HPC, 128, D).transpose(1, 0, 2).reshape(128, HPC * D)
        in_maps.append({
            "xT": _bf16(xt),
            "wqT": _bf16(wqt),
            "wkT": _bf16(wkt),
            "wvT": _bf16(wvt),
            "woT": _bf16(wot),
        })
    return in_maps


def combine_outputs(results, bo):
    bo = np.asarray(bo, dtype=np.float32)
    out = np.zeros((B, S, D), dtype=np.float32)
    for c in range(NCORES):
        b = c // (NCORES // B)
        out[b] += np.asarray(results[c]["out"], dtype=np.float32)
    out += bo[None, None, :]
    return out


def kernel(x, wq, wk, wv, wo, bo):
    nc = _get_built()
    in_maps = make_in_maps(x, wq, wk, wv, wo)
    res = run_bass_kernel_spmd(nc, in_maps, core_ids=list(range(NCORES)))
    return combine_outputs(res.results, bo)


if __name__ == "__main__":
    nc = _get_built()
    print("built ok; instructions:", len(nc.inst_map))



# revision 34
# speedup vs baseline: 1.0112x; 1.0057x over previous
"""Trainium2 Bass kernel for causal multi-head attention.

Problem: B=2, S=2048, D=2048, H=16 heads (HD=128), fp32, causal.
Sharding: 8 cores = 2 batches (data parallel) x 4 head-groups (tensor
parallel, 4 heads each). Each core computes Q/K/V projections for its
head slice, causal attention, and a partial out-projection; the host
sums the 4 partials per batch and adds the output bias.

Device layout notes:
  - All operands are bf16 (host pre-casts): every matmul runs at
    1 cycle/row at any moving size, DMA bytes are halved vs fp32, and
    PSUM accumulation stays fp32 so contraction precision is kept.
  - Scores are computed transposed (scores^T[k, q]) so the AV matmul
    uses V in natural [s, d] layout as the stationary operand,
    accumulating ctx^T[d, q] in PSUM over k-tiles.
  - Softmax denominators: exp tiles are accumulated over k-tiles into
    a bf16 SBUF accumulator on the DVE (16-bit DVE fast mode), then a
    single ones-vector matmul per (head, q-chunk) does the 128-way
    partition sum in fp32 PSUM. This removes the per-k-tile denominator
    matmuls from the PE (the bottleneck engine); the bf16 partials cost
    ~1e-3 relative on the denominator since the 128-way sum stays fp32.
  - exp() runs unnormalized (scores are O(6), no max subtraction);
    normalization happens once on ctx^T via a GPSIMD partition
    broadcast of the reciprocal denominators (the idle Pool engine),
    which frees a PSUM bank so the scores pool gets a third buffer
    (deeper PE lookahead over the exp latency).
  - Phase 2/3 is software-pipelined for the in-order PE: score matmuls
    run LAG=2 blocks ahead of the AV matmuls (so the ACT exp latency
    never stalls the PE), the previous chunk's out-projection matmuls
    are paced as fillers between AV emissions, and each head's
    denominator matmul is deferred one block into the next head.
    A deep pt ring (PTB=12) keeps the exp->mask/dacc->AV chain from
    throttling on tile reuse (6 -> 12 measured -35us with the rest).
  - DMA is issued as a few large slab transfers from host-pre-tiled
    DRAM layouts (make_in_maps packs x/w so each transfer is a plain
    2D slice with 4-16KB contiguous per-partition runs), amortizing
    the ~0.6us per-DMA HWDGE issue cost and minimizing descriptors.
  - Measured dead ends (this HW): stationary-weight reuse across
    matmuls (LDWEIGHTS already hidden by the PE reorder window),
    offloading dacc adds / masks to Pool (software Q7 loop, 2.4x
    worse), diagonal den partials as PE ones-matmuls (+21us), fp8
    anywhere in the signal path (e4m3 ~2.4% rms per operand vs the
    2e-2 gate).
"""

import sys

if "/opt/trn_rl_repo" not in sys.path:
    sys.path.insert(0, "/opt/trn_rl_repo")

import numpy as np

import concourse.bacc as bacc
import concourse.mybir as mybir
import concourse.tile as tile
from concourse.bass_utils import run_bass_kernel_spmd
from concourse.masks import make_upper_triangular

B, S, D, H = 2, 2048, 2048, 16
HD = 128                 # head dim
NCORES = 8
HPC = 4                  # heads per core
DC = HPC * HD            # 512: per-core projection width
CT = D // 128            # 16 contraction tiles
QT = S // 512            # 4 query chunks of 512
ST = S // 128            # 16 seq tiles of 128
SCALE = 1.0 / float(np.sqrt(HD))
F32 = mybir.dt.float32
BF16 = mybir.dt.bfloat16
EXP = mybir.ActivationFunctionType.Exp

_BUILT = None


def _build(cfg=None, reps=1):
    cfg = cfg or {}
    STAGE = cfg.get("stage", 3)   # 1: proj only; 2: +attention; 3: full
    SLIM = cfg.get("slimdma", 0)  # >0: truncate input DMAs (timing diag only)
    NONORM = cfg.get("nonorm", 0)  # skip softmax denominator (timing diag only)
    XCB = cfg.get("xcb", 2)    # x chunk slab bufs
    PTB = cfg.get("ptb", 12)   # p^T tile bufs (deep ring: pt lifetime spans
                               # exp -> mask/dacc (DVE queue) -> AV matmul)
    SCB = cfg.get("scb", 3)    # scores psum bufs
    CPB = cfg.get("cpb", 2)    # ctx psum bufs
    PPB = cfg.get("ppb", 2)    # proj psum bufs (per m-tag)
    DAB = cfg.get("dab", 2)    # den accumulator (sbuf) bufs
    OTB = cfg.get("otb", 3)    # out sbuf slab bufs
    nc = bacc.Bacc(trn_type="TRN2", target_bir_lowering=False)
    # inputs are host-pre-tiled so every DMA is a plain 2D slice whose
    # per-partition runs are 4-16KB contiguous (few large descriptors):
    #   xT:  [(n g p), (ct_in_g c)] with x[b,s,d] at [n*512+g*128... see
    #        make_in_maps; slab (n,g) is rows (n*4+g)*128..+128, all cols
    #   wq/wk/wv: [p, (ct, dc)];  wo: [p, (i, d)]
    xT_d = nc.dram_tensor("xT", [QT * 4 * 128, 4 * 512], BF16, kind="ExternalInput")
    wqT_d = nc.dram_tensor("wqT", [128, CT * DC], BF16, kind="ExternalInput")
    wkT_d = nc.dram_tensor("wkT", [128, CT * DC], BF16, kind="ExternalInput")
    wvT_d = nc.dram_tensor("wvT", [128, CT * DC], BF16, kind="ExternalInput")
    woT_d = nc.dram_tensor("woT", [128, HPC * D], BF16, kind="ExternalInput")
    out_d = nc.dram_tensor("out", [S, D], BF16, kind="ExternalOutput")

    with tile.TileContext(nc) as tc:
      for _rep in range(reps):
        _p = f"r{_rep}_"
        with (
            tc.tile_pool(name=_p + "const", bufs=1) as cst,
            tc.tile_pool(name=_p + "persist", bufs=1) as pp,
        ):
            # upper-triangular (incl diagonal) 0/1 mask: allowed = k <= q
            tri_f = cst.tile([128, 128], F32, tag="tri_f", name="tri_f")
            make_upper_triangular(nc, tri_f[:], val=1.0, diag=True)
            tri = cst.tile([128, 128], BF16, tag="tri", name="tri")
            nc.vector.tensor_copy(tri[:], tri_f[:])
            ones_col = cst.tile([128, 1], BF16, tag="ones_col", name="ones_col")
            nc.vector.memset(ones_col[:], 1.0)

            # persistent per-core tensors (partition dim x free dim):
            # qT/kT: per head [HD, S]; v: per s-tile [128, DC]; ctx^T per
            # (head, q-chunk) for fine-grained deps so the out-projection of
            # chunk qt can overlap attention of chunk qt+1
            qTt = [pp.tile([128, S], BF16, tag=f"qT{h}", name=f"qT{h}") for h in range(HPC)]
            kTt = [pp.tile([128, S], BF16, tag=f"kT{h}", name=f"kT{h}") for h in range(HPC)]
            vt = [pp.tile([128, DC], BF16, tag=f"v{s}", name=f"v{s}") for s in range(ST)]
            ctxt = [[pp.tile([128, 512], BF16, tag=f"ctx{h}_{q}", name=f"ctx{h}_{q}")
                     for q in range(QT)] for h in range(HPC)]

            # resident weights, one slab DMA each:
            #   wq/wk/wv: [128, (ct, dc)]  <- [D, DC] DRAM
            #   wo:       [128, (i, d)]    <- [DC, D] DRAM
            wq_sb = pp.tile([128, CT * DC], BF16, tag="wq_sb", name="wq_sb")
            wk_sb = pp.tile([128, CT * DC], BF16, tag="wk_sb", name="wk_sb")
            wv_sb = pp.tile([128, CT * DC], BF16, tag="wv_sb", name="wv_sb")
            wo_sb = pp.tile([128, HPC * D], BF16, tag="wo_sb", name="wo_sb")

            # ---------------- Phase 1: Q/K/V projections ----------------
            with (
                tc.tile_pool(name=_p + "xc", bufs=XCB) as xcp,
                tc.tile_pool(name=_p + "proj_psum", bufs=PPB, space="PSUM") as pps,
            ):
                for n in range(QT):  # s-chunks of 512
                    # x chunk in 4 ct-group sub-slabs so the first matmuls of
                    # chunk 0 can start ~3us in instead of waiting ~25us for
                    # serialized whole-slab DMAs. For n==0 the wq sub-slabs
                    # are interleaved with the x sub-slabs (Q needs both);
                    # wk/wv/wo follow (K/V matmuls run later).
                    xc = xcp.tile([128, CT * 512], BF16, tag="xc", name=f"xc_{n}")
                    # the first chunk's leading sub-slabs are quartered so the
                    # first Q matmuls start earlier
                    gsplit = 4 if n == 0 else 1
                    for g in range(4):
                        r0 = (n * 4 + g) * 128
                        for s in range(gsplit if g == 0 else 1):
                            w = 2048 // (gsplit if g == 0 else 1)
                            we = min(w, SLIM) if SLIM else w
                            nc.sync.dma_start(
                                out=xc[:, g * 2048 + s * w:g * 2048 + s * w + we],
                                in_=xT_d[r0:r0 + 128, s * w:s * w + we],
                            )
                            if n == 0:
                                ww = 4 * DC // (gsplit if g == 0 else 1)
                                wwe = min(ww, SLIM) if SLIM else ww
                                nc.sync.dma_start(
                                    out=wq_sb[:, g * 4 * DC + s * ww:
                                              g * 4 * DC + s * ww + wwe],
                                    in_=wqT_d[:, g * 4 * DC + s * ww:
                                              g * 4 * DC + s * ww + wwe],
                                )
                    if n == 0:
                        # wk split in 4 so K's ct-progressive needs are met
                        # without waiting behind one monolithic transfer;
                        # wv/wo later (V/out-proj matmuls run much later)
                        qw = CT * DC // 4
                        for g4 in range(4):
                            sl = slice(g4 * qw, g4 * qw + (min(qw, SLIM) if SLIM else qw))
                            nc.sync.dma_start(out=wk_sb[:, sl], in_=wkT_d[:, sl])
                        for g2 in range(2):
                            hw_ = CT * DC // 2
                            sl = slice(g2 * hw_, g2 * hw_ + (min(hw_, SLIM) if SLIM else hw_))
                            nc.sync.dma_start(out=wv_sb[:, sl], in_=wvT_d[:, sl])
                        if SLIM:
                            nc.sync.dma_start(out=wo_sb[:, :SLIM], in_=woT_d[:, :SLIM])
                        else:
                            nc.sync.dma_start(out=wo_sb[:], in_=woT_d[:])

                    # Q^T and K^T: out[d-tile(=head) 128, s 512] accum over ct.
                    # For n>=1 the x slab is already resident, so run each
                    # head's 16-matmul accumulation bank-contiguously (m outer)
                    # — consecutive matmuls hit the same PSUM bank and the
                    # drain copy issues right behind each group. n==0 must
                    # stay ct-outer to track the progressive x/w DMA arrival.
                    for w_sb, dst in ((wq_sb, qTt), (wk_sb, kTt)):
                        acc = [pps.tile([128, 512], F32, tag=f"acc{m}", name=f"acc_{n}_{m}")
                               for m in range(HPC)]

                        def _qk_mm(ct, m, w_sb=w_sb, acc=acc, xc=xc):
                            nc.tensor.matmul(
                                acc[m][:],
                                (w_sb[:, ct * DC + m * 128:ct * DC + (m + 1) * 128]),
                                (xc[:, ct * 512:(ct + 1) * 512]),
                                start=(ct == 0),
                                stop=(ct == CT - 1),
                            )

                        def _qk_copy(m, dst=dst, acc=acc, n=n):
                            eng = nc.vector if (m % 2 == 0) else nc.scalar
                            if eng is nc.vector:
                                eng.tensor_copy(dst[m][:, n * 512:(n + 1) * 512], acc[m][:])
                            else:
                                eng.copy(dst[m][:, n * 512:(n + 1) * 512], acc[m][:])

                        if n == 0:
                            for ct in range(CT):
                                for m in range(HPC):
                                    _qk_mm(ct, m)
                            for m in range(HPC):
                                _qk_copy(m)
                        else:
                            for m in range(HPC):
                                for ct in range(CT):
                                    _qk_mm(ct, m)
                                _qk_copy(m)

                    # V natural [s-tile 128, d 512]: lhsT = x^T chunk slice
                    accv = [pps.tile([128, 512], F32, tag=f"acc{ss}", name=f"accv_{n}_{ss}")
                            for ss in range(4)]
                    for ss in range(4):
                        for ct in range(CT):
                            nc.tensor.matmul(
                                accv[ss][:],
                                (xc[:, ct * 512 + ss * 128:ct * 512 + (ss + 1) * 128]),
                                (wv_sb[:, ct * DC:(ct + 1) * DC]),
                                start=(ct == 0),
                                stop=(ct == CT - 1),
                            )
                        eng = nc.vector if (ss % 2 == 0) else nc.scalar
                        if eng is nc.vector:
                            eng.tensor_copy(vt[n * 4 + ss][:], accv[ss][:])
                        else:
                            eng.copy(vt[n * 4 + ss][:], accv[ss][:])

            if STAGE == 1:
                # timing diag: projections only
                for h in range(HPC):
                    nc.sync.dma_start(out=out_d[h * 128:(h + 1) * 128, :],
                                      in_=qTt[h][:])
                continue

            # ------- Phase 2+3: causal attention with interleaved out-proj ----
            # The PE executes in program order, so the naive per-block order
            # (score MM -> exp on ACT -> AV MM) stalls the PE ~500ns per block
            # waiting for its own exp. Software-pipeline instead: emit score
            # MMs LAG blocks ahead of AV MMs, and pace the PREVIOUS chunk's
            # out-projection matmuls as fillers between AV emissions so the PE
            # always has ready work while ACT catches up. Out-proj PSUM drains
            # go to the otherwise-idle Pool engine (ACT is exp-saturated in
            # this phase).
            with (
                tc.tile_pool(name=_p + "ptp", bufs=PTB) as ptp,
                tc.tile_pool(name=_p + "dap", bufs=DAB) as dap,
                tc.tile_pool(name=_p + "rcp", bufs=2) as rcp,
                tc.tile_pool(name=_p + "rbs", bufs=2) as rbsp,
                tc.tile_pool(name=_p + "osb", bufs=OTB) as osp,
                tc.tile_pool(name=_p + "sc_ps", bufs=SCB, space="PSUM") as scp,
                tc.tile_pool(name=_p + "ctx_ps", bufs=CPB, space="PSUM") as cxp,
                tc.tile_pool(name=_p + "den_ps", bufs=1, space="PSUM") as dnp,
                tc.tile_pool(name=_p + "out_ps", bufs=1, space="PSUM") as ops,
            ):
                LAG = cfg.get("lag", 2)  # AV MM trails its score MM by LAG blocks

                def make_outproj_thunks(qt, ctx_q):
                    """64 thunks, one PE matmul each; PSUM drains on Pool and
                    the out DMA ride along with the closing matmul of a group."""
                    thunks = []
                    state = {}
                    for r in range(4):
                        q = qt * 4 + r
                        for oc in range(4):
                            for i in range(HPC):
                                def t(qt=qt, q=q, r=r, oc=oc, i=i, ctx_q=ctx_q):
                                    if oc == 0 and i == 0:
                                        state["ot"] = osp.tile(
                                            [128, D], BF16, tag="ot", name=f"ot_{q}")
                                    if i == 0:
                                        state["po"] = ops.tile(
                                            [128, 512], F32, tag=f"po{oc % 2}",
                                            name=f"po_{q}_{oc}")
                                    nc.tensor.matmul(
                                        state["po"][:],
                                        (ctx_q[i][:, r * 128:(r + 1) * 128]),
                                        (wo_sb[:, i * D + oc * 512:i * D + (oc + 1) * 512]),
                                        start=(i == 0),
                                        stop=(i == HPC - 1),
                                    )
                                    if i == HPC - 1:
                                        # Pool can't read PSUM; split drains
                                        # 5:3 DVE:ACT (ACT carries the exps)
                                        if (r * 4 + oc) % 8 < 5:
                                            nc.vector.tensor_copy(
                                                state["ot"][:, oc * 512:(oc + 1) * 512],
                                                state["po"][:])
                                        else:
                                            nc.scalar.copy(
                                                state["ot"][:, oc * 512:(oc + 1) * 512],
                                                state["po"][:])
                                        if qt == QT - 1 and r == 3:
                                            # kernel tail: ship each quarter as
                                            # soon as it drains
                                            nc.sync.dma_start(
                                                out=out_d[q * 128:(q + 1) * 128,
                                                          oc * 512:(oc + 1) * 512],
                                                in_=state["ot"][:, oc * 512:(oc + 1) * 512])
                                        elif oc == 3:
                                            nc.sync.dma_start(
                                                out=out_d[q * 128:(q + 1) * 128, :],
                                                in_=state["ot"][:])
                                thunks.append(t)
                    return thunks

                pending = []   # out-proj thunks from the previous chunk
                pend_i = 0

                for qt in range(QT):
                    nkt = 4 * qt + 4  # causal: k-tiles 0..4qt+3
                    n_slots = nkt * HPC  # av-emission slots this chunk
                    slot = 0

                    def fillers():
                        # keep pending consumption proportional to progress,
                        # holding back a few thunks to cover the last head's
                        # den chain and the next chunk's out-proj warmup
                        nonlocal pend_i
                        if not pending:
                            return
                        avail = max(0, len(pending) - 6)
                        target = (avail * slot + n_slots - 1) // n_slots
                        while pend_i < min(target, avail):
                            pending[pend_i]()
                            pend_i += 1

                    # NOTE: offloading dacc adds / mask muls to Pool measured
                    # 2.4x WORSE on HW (gpsimd is a software Q7 loop, ~2x DVE
                    # cost per op, and it serialized the phase) — keep Pool to
                    # the 16 partition broadcasts only.
                    ctx_q = []  # per-head normalized ctx^T [128, 512] tiles
                    deferred_den = None
                    for h in range(HPC):
                        cps = cxp.tile([128, 512], F32, tag="cps", name=f"cps_{h}_{qt}")
                        # bf16 den accumulator on the DVE; the 128-way k sum
                        # happens once per (h, qt) in fp32 PSUM below, so the
                        # bf16 partial rounding stays ~1e-3 on the denominator.
                        # (Folding the diagonal-tile partials on the PE as
                        # ones-matmuls instead measured +21us: tiny matmuls
                        # disrupt the PE stream more than they relieve DVE.)
                        dacc = dap.tile([128, 512], BF16, tag="dacc", name=f"dacc_{h}_{qt}")
                        pts = {}
                        los = {}

                        def emit_av(kt, cps=cps, h=h, nkt=nkt, pts=pts, los=los):
                            nc.tensor.matmul(
                                cps[:, los[kt]:],
                                (vt[kt][:, h * 128:(h + 1) * 128]),
                                (pts[kt][:, los[kt]:]),
                                start=(kt == 0), stop=(kt == nkt - 1),
                            )

                        for kt in range(nkt):
                            j = kt - 4 * qt
                            # For diagonal blocks only q-cols >= 128j are
                            # unmasked; shrink the matmul N-range to skip the
                            # masked region instead of zero-filling it.
                            lo = 0 if j < 0 else j * 128
                            los[kt] = lo
                            sc = scp.tile([128, 512], F32, tag="sc", name=f"sc_{h}_{qt}_{kt}")
                            nc.tensor.matmul(
                                sc[:, lo:],
                                (kTt[h][:, kt * 128:(kt + 1) * 128]),
                                (qTt[h][:, qt * 512 + lo:(qt + 1) * 512]),
                                start=True,
                                stop=True,
                            )
                            # previous head's denominator matmul slots in here,
                            # one block after its dacc completed (no PE stall)
                            if kt == 1 and deferred_den is not None:
                                deferred_den()
                                deferred_den = None
                            pt = ptp.tile([128, 512], BF16, tag="pt", name=f"pt_{h}_{qt}_{kt}")
                            pts[kt] = pt
                            nc.scalar.activation(
                                pt[:, lo:], sc[:, lo:], EXP, scale=SCALE
                            )
                            if j >= 0:
                                # strictly-diagonal 128x128 sub-block mask
                                nc.vector.tensor_mul(
                                    pt[:, j * 128:(j + 1) * 128],
                                    pt[:, j * 128:(j + 1) * 128],
                                    tri[:],
                                )
                            with nc.allow_low_precision("bf16 den partials; final 128-way sum is fp32 in PSUM"):
                                if NONORM:
                                    pass
                                elif kt == 0:
                                    nc.vector.tensor_copy(dacc[:], pt[:])
                                else:
                                    nc.vector.tensor_add(
                                        dacc[:, lo:], dacc[:, lo:], pt[:, lo:])
                            if kt >= LAG:
                                emit_av(kt - LAG)
                                slot += 1
                                fillers()
                        for kt in range(max(0, nkt - LAG), nkt):
                            emit_av(kt)
                            slot += 1
                            fillers()

                        def make_den(h=h, qt=qt, dacc=dacc, cps=cps):
                            def den_thunk():
                                if NONORM:  # timing diag: plain PSUM drain
                                    ctx = ctxt[h][qt]
                                    nc.vector.tensor_copy(ctx[:], cps[:])
                                    ctx_q.append(ctx)
                                    return
                                den = dnp.tile([1, 512], F32, tag="den", name=f"den_{h}_{qt}")
                                nc.tensor.matmul(
                                    den[:], (ones_col[:]), (dacc[:]),
                                    start=True, stop=True,
                                )
                                recip = rcp.tile([1, 512], F32, tag="recip", name=f"recip_{h}_{qt}")
                                # ~5x faster than reciprocal(); 18-bit accuracy
                                # is plenty for the softmax denominator and the
                                # den range (>=1, <<1e38) avoids the edge cases
                                nc.vector.reciprocal_approx_fast(recip[:], den[:])
                                # reciprocal broadcast on the Pool engine frees
                                # a PSUM bank (no PE broadcast matmul)
                                rbs = rbsp.tile([128, 512], F32, tag="rbs", name=f"rbs_{h}_{qt}")
                                nc.gpsimd.partition_broadcast(rbs[:], recip[:])
                                ctx = ctxt[h][qt]
                                nc.vector.tensor_mul(ctx[:], cps[:], rbs[:])
                                ctx_q.append(ctx)
                            return den_thunk

                        deferred_den = make_den()

                    # last head's den: give the dacc adds a moment by draining
                    # a couple of fillers first
                    if pending and pend_i < len(pending):
                        pending[pend_i]()
                        pend_i += 1
                        if pend_i < len(pending):
                            pending[pend_i]()
                            pend_i += 1
                    if deferred_den is not None:
                        deferred_den()
                        deferred_den = None

                    # flush any remaining fillers, then queue this chunk's
                    # out-projection for interleaving into the next chunk
                    while pend_i < len(pending):
                        pending[pend_i]()
                        pend_i += 1
                    if STAGE >= 3:
                        pending = make_outproj_thunks(qt, ctx_q)
                        pend_i = 0
                    elif qt == QT - 1:
                        # timing diag: attention only; drain ctx tiles
                        for i, ctx in enumerate(ctx_q):
                            nc.sync.dma_start(
                                out=out_d[i * 128:(i + 1) * 128, :512],
                                in_=ctx[:])

                # last chunk's out-projection runs as a straight PE stream
                while pend_i < len(pending):
                    pending[pend_i]()
                    pend_i += 1

    nc.compile()
    return nc


def _get_built():
    global _BUILT
    if _BUILT is None:
        _BUILT = _build()
    return _BUILT


def _bf16(a):
    import ml_dtypes
    return np.ascontiguousarray(a).astype(ml_dtypes.bfloat16)


def make_in_maps(x, wq, wk, wv, wo):
    x = np.asarray(x, dtype=np.float32)
    wq = np.asarray(wq, dtype=np.float32)
    wk = np.asarray(wk, dtype=np.float32)
    wv = np.asarray(wv, dtype=np.float32)
    wo = np.asarray(wo, dtype=np.float32)
    in_maps = []
    for c in range(NCORES):
        b, hg = divmod(c, NCORES // B)
        sl = slice(hg * DC, (hg + 1) * DC)
        # pre-tile for dense-descriptor DMA (see _build dram layout notes)
        xt = x[b].T.reshape(4, 4, 128, 4, 512)          # [g, ct', p, n, c]
        xt = xt.transpose(3, 0, 2, 1, 4).reshape(QT * 4 * 128, 4 * 512)
        wqt = wq[sl, :].T.reshape(CT, 128, DC).transpose(1, 0, 2).reshape(128, CT * DC)
        wkt = wk[sl, :].T.reshape(CT, 128, DC).transpose(1, 0, 2).reshape(128, CT * DC)
        wvt = wv[sl, :].T.reshape(CT, 128, DC).transpose(1, 0, 2).reshape(128, CT * DC)
        wot = wo[:, sl].T.reshape(HPC, 128, D).transpose(1, 0, 2).reshape(128, HPC * D)
        in_maps.append({
            "xT": _bf16(xt),
            "wqT": _bf16(wqt),
            "wkT": _bf16(wkt),
            "wvT": _bf16(wvt),
            "woT": _bf16(wot),
        })
    return in_maps


def combine_outputs(results, bo):
    bo = np.asarray(bo, dtype=np.float32)
    out = np.zeros((B, S, D), dtype=np.float32)
    for c in range(NCORES):
        b = c // (NCORES // B)
        out[b] += np.asarray(results[c]["out"], dtype=np.float32)
    out += bo[None, None, :]
    return out


def kernel(x, wq, wk, wv, wo, bo):
    nc = _get_built()
    in_maps = make_in_maps(x, wq, wk, wv, wo)
    res = run_bass_kernel_spmd(nc, in_maps, core_ids=list(range(NCORES)))
    return combine_outputs(res.results, bo)


if __name__ == "__main__":
    nc = _get_built()
    print("built ok; instructions:", len(nc.inst_map))



# revision 36
# speedup vs baseline: 1.0472x; 1.0356x over previous
"""Trainium2 Bass kernel for causal multi-head attention.

Problem: B=2, S=2048, D=2048, H=16 heads (HD=128), fp32, causal.
Sharding: 8 cores = 2 batches (data parallel) x 4 head-groups (tensor
parallel, 4 heads each). Each core computes Q/K/V projections for its
head slice, causal attention, and a partial out-projection; the host
sums the 4 partials per batch and adds the output bias.

Device layout notes:
  - All operands are bf16 (host pre-casts): every matmul runs at
    1 cycle/row at any moving size, DMA bytes are halved vs fp32, and
    PSUM accumulation stays fp32 so contraction precision is kept.
  - Scores are computed transposed (scores^T[k, q]) so the AV matmul
    uses V in natural [s, d] layout as the stationary operand,
    accumulating ctx^T[d, q] in PSUM over k-tiles.
  - Softmax denominators: exp tiles are accumulated over k-tiles into
    a bf16 SBUF accumulator on the DVE (16-bit DVE fast mode), then a
    single ones-vector matmul per (head, q-chunk) does the 128-way
    partition sum in fp32 PSUM. This removes the per-k-tile denominator
    matmuls from the PE (the bottleneck engine); the bf16 partials cost
    ~1e-3 relative on the denominator since the 128-way sum stays fp32.
  - exp() runs unnormalized (scores are O(6), no max subtraction);
    normalization happens once on ctx^T via a GPSIMD partition
    broadcast of the reciprocal denominators (the idle Pool engine),
    which frees a PSUM bank so the scores pool gets a third buffer
    (deeper PE lookahead over the exp latency).
  - Phase 2/3 is software-pipelined for the in-order PE: score matmuls
    run LAG=2 blocks ahead of the AV matmuls (so the ACT exp latency
    never stalls the PE), the previous chunk's out-projection matmuls
    are paced as fillers between AV emissions, and each head's
    denominator matmul is deferred one block into the next head.
    A deep pt ring (PTB=12) keeps the exp->mask/dacc->AV chain from
    throttling on tile reuse (6 -> 12 measured -35us with the rest).
  - DMA is issued as a few large slab transfers from host-pre-tiled
    DRAM layouts (make_in_maps packs x/w so each transfer is a plain
    2D slice with 4-16KB contiguous per-partition runs), amortizing
    the ~0.6us per-DMA HWDGE issue cost and minimizing descriptors.
  - Measured dead ends (this HW): stationary-weight reuse across
    matmuls (LDWEIGHTS already hidden by the PE reorder window),
    offloading dacc adds / masks to Pool (software Q7 loop, 2.4x
    worse), diagonal den partials as PE ones-matmuls (+21us), fp8
    anywhere in the signal path (e4m3 ~2.4% rms per operand vs the
    2e-2 gate).
"""

import sys

if "/opt/trn_rl_repo" not in sys.path:
    sys.path.insert(0, "/opt/trn_rl_repo")

import numpy as np

import concourse.bacc as bacc
import concourse.mybir as mybir
import concourse.tile as tile
from concourse.bass_utils import run_bass_kernel_spmd
from concourse.masks import make_upper_triangular

B, S, D, H = 2, 2048, 2048, 16
HD = 128                 # head dim
NCORES = 8
HPC = 4                  # heads per core
DC = HPC * HD            # 512: per-core projection width
CT = D // 128            # 16 contraction tiles
QT = S // 512            # 4 query chunks of 512
ST = S // 128            # 16 seq tiles of 128
SCALE = 1.0 / float(np.sqrt(HD))
F32 = mybir.dt.float32
BF16 = mybir.dt.bfloat16
EXP = mybir.ActivationFunctionType.Exp

_BUILT = None


def _build(cfg=None, reps=1):
    cfg = cfg or {}
    STAGE = cfg.get("stage", 3)   # 1: proj only; 2: +attention; 3: full
    SLIM = cfg.get("slimdma", 0)  # >0: truncate input DMAs (timing diag only)
    NONORM = cfg.get("nonorm", 0)  # skip softmax denominator (timing diag only)
    XCB = cfg.get("xcb", 2)    # x chunk slab bufs
    PTB = cfg.get("ptb", 16)   # p^T tile bufs (deep ring: pt lifetime spans
                               # exp -> mask/dacc (DVE queue) -> AV matmul)
    SCB = cfg.get("scb", 3)    # scores psum bufs
    CPB = cfg.get("cpb", 2)    # ctx psum bufs
    PPB = cfg.get("ppb", 2)    # proj psum bufs (per m-tag)
    DAB = cfg.get("dab", 3)    # den accumulator (sbuf) bufs
    OTB = cfg.get("otb", 4)    # out sbuf slab bufs
    nc = bacc.Bacc(trn_type="TRN2", target_bir_lowering=False)
    # inputs are host-pre-tiled so every DMA is a plain 2D slice whose
    # per-partition runs are 4-16KB contiguous (few large descriptors):
    #   xT:  [(n g p), (ct_in_g c)] with x[b,s,d] at [n*512+g*128... see
    #        make_in_maps; slab (n,g) is rows (n*4+g)*128..+128, all cols
    #   wq/wk/wv: [p, (ct, dc)];  wo: [p, (i, d)]
    xT_d = nc.dram_tensor("xT", [QT * 4 * 128, 4 * 512], BF16, kind="ExternalInput")
    wqT_d = nc.dram_tensor("wqT", [128, CT * DC], BF16, kind="ExternalInput")
    wkT_d = nc.dram_tensor("wkT", [128, CT * DC], BF16, kind="ExternalInput")
    wvT_d = nc.dram_tensor("wvT", [128, CT * DC], BF16, kind="ExternalInput")
    woT_d = nc.dram_tensor("woT", [128, HPC * D], BF16, kind="ExternalInput")
    out_d = nc.dram_tensor("out", [S, D], BF16, kind="ExternalOutput")

    with tile.TileContext(nc) as tc:
      for _rep in range(reps):
        _p = f"r{_rep}_"
        with (
            tc.tile_pool(name=_p + "const", bufs=1) as cst,
            tc.tile_pool(name=_p + "persist", bufs=1) as pp,
        ):
            # upper-triangular (incl diagonal) 0/1 mask: allowed = k <= q
            tri_f = cst.tile([128, 128], F32, tag="tri_f", name="tri_f")
            make_upper_triangular(nc, tri_f[:], val=1.0, diag=True)
            tri = cst.tile([128, 128], BF16, tag="tri", name="tri")
            nc.vector.tensor_copy(tri[:], tri_f[:])
            ones_col = cst.tile([128, 1], BF16, tag="ones_col", name="ones_col")
            nc.vector.memset(ones_col[:], 1.0)

            # persistent per-core tensors (partition dim x free dim):
            # qT/kT: per head [HD, S]; v: per s-tile [128, DC]; ctx^T per
            # (head, q-chunk) for fine-grained deps so the out-projection of
            # chunk qt can overlap attention of chunk qt+1
            qTt = [pp.tile([128, S], BF16, tag=f"qT{h}", name=f"qT{h}") for h in range(HPC)]
            kTt = [pp.tile([128, S], BF16, tag=f"kT{h}", name=f"kT{h}") for h in range(HPC)]
            vt = [pp.tile([128, DC], BF16, tag=f"v{s}", name=f"v{s}") for s in range(ST)]
            ctxt = [[pp.tile([128, 512], BF16, tag=f"ctx{h}_{q}", name=f"ctx{h}_{q}")
                     for q in range(QT)] for h in range(HPC)]

            # resident weights, one slab DMA each:
            #   wq/wk/wv: [128, (ct, dc)]  <- [D, DC] DRAM
            #   wo:       [128, (i, d)]    <- [DC, D] DRAM
            wq_sb = pp.tile([128, CT * DC], BF16, tag="wq_sb", name="wq_sb")
            wk_sb = pp.tile([128, CT * DC], BF16, tag="wk_sb", name="wk_sb")
            wv_sb = pp.tile([128, CT * DC], BF16, tag="wv_sb", name="wv_sb")
            wo_sb = pp.tile([128, HPC * D], BF16, tag="wo_sb", name="wo_sb")

            # ---------------- Phase 1: Q/K/V projections ----------------
            with (
                tc.tile_pool(name=_p + "xc", bufs=XCB) as xcp,
                tc.tile_pool(name=_p + "proj_psum", bufs=PPB, space="PSUM") as pps,
            ):
                for n in range(QT):  # s-chunks of 512
                    # x chunk in 4 ct-group sub-slabs so the first matmuls of
                    # chunk 0 can start ~3us in instead of waiting ~25us for
                    # serialized whole-slab DMAs. For n==0 the wq sub-slabs
                    # are interleaved with the x sub-slabs (Q needs both);
                    # wk/wv/wo follow (K/V matmuls run later).
                    xc = xcp.tile([128, CT * 512], BF16, tag="xc", name=f"xc_{n}")
                    # the first chunk's leading sub-slabs are quartered so the
                    # first Q matmuls start earlier
                    gsplit = 4 if n == 0 else 1
                    for g in range(4):
                        r0 = (n * 4 + g) * 128
                        for s in range(gsplit if g == 0 else 1):
                            w = 2048 // (gsplit if g == 0 else 1)
                            we = min(w, SLIM) if SLIM else w
                            nc.sync.dma_start(
                                out=xc[:, g * 2048 + s * w:g * 2048 + s * w + we],
                                in_=xT_d[r0:r0 + 128, s * w:s * w + we],
                            )
                            if n == 0:
                                ww = 4 * DC // (gsplit if g == 0 else 1)
                                wwe = min(ww, SLIM) if SLIM else ww
                                nc.sync.dma_start(
                                    out=wq_sb[:, g * 4 * DC + s * ww:
                                              g * 4 * DC + s * ww + wwe],
                                    in_=wqT_d[:, g * 4 * DC + s * ww:
                                              g * 4 * DC + s * ww + wwe],
                                )
                    if n == 0:
                        # wk split in 4 so K's ct-progressive needs are met
                        # without waiting behind one monolithic transfer;
                        # wv/wo later (V/out-proj matmuls run much later)
                        qw = CT * DC // 4
                        for g4 in range(4):
                            sl = slice(g4 * qw, g4 * qw + (min(qw, SLIM) if SLIM else qw))
                            nc.sync.dma_start(out=wk_sb[:, sl], in_=wkT_d[:, sl])
                        for g2 in range(2):
                            hw_ = CT * DC // 2
                            sl = slice(g2 * hw_, g2 * hw_ + (min(hw_, SLIM) if SLIM else hw_))
                            nc.sync.dma_start(out=wv_sb[:, sl], in_=wvT_d[:, sl])
                        if SLIM:
                            nc.sync.dma_start(out=wo_sb[:, :SLIM], in_=woT_d[:, :SLIM])
                        else:
                            nc.sync.dma_start(out=wo_sb[:], in_=woT_d[:])

                    # Q^T and K^T: out[d-tile(=head) 128, s 512] accum over ct
                    for w_sb, dst in ((wq_sb, qTt), (wk_sb, kTt)):
                        acc = [pps.tile([128, 512], F32, tag=f"acc{m}", name=f"acc_{n}_{m}")
                               for m in range(HPC)]
                        for ct in range(CT):
                            for m in range(HPC):
                                nc.tensor.matmul(
                                    acc[m][:],
                                    (w_sb[:, ct * DC + m * 128:ct * DC + (m + 1) * 128]),
                                    (xc[:, ct * 512:(ct + 1) * 512]),
                                    start=(ct == 0),
                                    stop=(ct == CT - 1),
                                )
                        for m in range(HPC):
                            eng = nc.vector if (m % 2 == 0) else nc.scalar
                            if eng is nc.vector:
                                eng.tensor_copy(dst[m][:, n * 512:(n + 1) * 512], acc[m][:])
                            else:
                                eng.copy(dst[m][:, n * 512:(n + 1) * 512], acc[m][:])

                    # V natural [s-tile 128, d 512]: lhsT = x^T chunk slice
                    accv = [pps.tile([128, 512], F32, tag=f"acc{ss}", name=f"accv_{n}_{ss}")
                            for ss in range(4)]
                    for ct in range(CT):
                        for ss in range(4):
                            nc.tensor.matmul(
                                accv[ss][:],
                                (xc[:, ct * 512 + ss * 128:ct * 512 + (ss + 1) * 128]),
                                (wv_sb[:, ct * DC:(ct + 1) * DC]),
                                start=(ct == 0),
                                stop=(ct == CT - 1),
                            )
                    for ss in range(4):
                        eng = nc.vector if (ss % 2 == 0) else nc.scalar
                        if eng is nc.vector:
                            eng.tensor_copy(vt[n * 4 + ss][:], accv[ss][:])
                        else:
                            eng.copy(vt[n * 4 + ss][:], accv[ss][:])

            if STAGE == 1:
                # timing diag: projections only
                for h in range(HPC):
                    nc.sync.dma_start(out=out_d[h * 128:(h + 1) * 128, :],
                                      in_=qTt[h][:])
                continue

            # ------- Phase 2+3: causal attention with interleaved out-proj ----
            # The PE executes in program order, so the naive per-block order
            # (score MM -> exp on ACT -> AV MM) stalls the PE ~500ns per block
            # waiting for its own exp. Software-pipeline instead: emit score
            # MMs LAG blocks ahead of AV MMs, and pace the PREVIOUS chunk's
            # out-projection matmuls as fillers between AV emissions so the PE
            # always has ready work while ACT catches up. Out-proj PSUM drains
            # go to the otherwise-idle Pool engine (ACT is exp-saturated in
            # this phase).
            with (
                tc.tile_pool(name=_p + "ptp", bufs=PTB) as ptp,
                tc.tile_pool(name=_p + "dap", bufs=DAB) as dap,
                tc.tile_pool(name=_p + "rcp", bufs=3) as rcp,
                tc.tile_pool(name=_p + "rbs", bufs=3) as rbsp,
                tc.tile_pool(name=_p + "osb", bufs=OTB) as osp,
                tc.tile_pool(name=_p + "sc_ps", bufs=SCB, space="PSUM") as scp,
                tc.tile_pool(name=_p + "ctx_ps", bufs=CPB, space="PSUM") as cxp,
                tc.tile_pool(name=_p + "den_ps", bufs=1, space="PSUM") as dnp,
                tc.tile_pool(name=_p + "out_ps", bufs=1, space="PSUM") as ops,
            ):
                LAG = cfg.get("lag", 2)  # AV MM trails its score MM by LAG blocks

                def make_outproj_thunks(qt, ctx_q):
                    """64 thunks, one PE matmul each; PSUM drains on Pool and
                    the out DMA ride along with the closing matmul of a group."""
                    thunks = []
                    state = {}
                    for r in range(4):
                        q = qt * 4 + r
                        for oc in range(4):
                            for i in range(HPC):
                                def t(qt=qt, q=q, r=r, oc=oc, i=i, ctx_q=ctx_q):
                                    if oc == 0 and i == 0:
                                        state["ot"] = osp.tile(
                                            [128, D], BF16, tag="ot", name=f"ot_{q}")
                                    if i == 0:
                                        state["po"] = ops.tile(
                                            [128, 512], F32, tag=f"po{oc % 2}",
                                            name=f"po_{q}_{oc}")
                                    nc.tensor.matmul(
                                        state["po"][:],
                                        (ctx_q[i][:, r * 128:(r + 1) * 128]),
                                        (wo_sb[:, i * D + oc * 512:i * D + (oc + 1) * 512]),
                                        start=(i == 0),
                                        stop=(i == HPC - 1),
                                    )
                                    if i == HPC - 1:
                                        # Pool can't read PSUM; split drains
                                        # 5:3 DVE:ACT (ACT carries the exps)
                                        if (r * 4 + oc) % 8 < 5:
                                            nc.vector.tensor_copy(
                                                state["ot"][:, oc * 512:(oc + 1) * 512],
                                                state["po"][:])
                                        else:
                                            nc.scalar.copy(
                                                state["ot"][:, oc * 512:(oc + 1) * 512],
                                                state["po"][:])
                                        if qt == QT - 1 and r == 3:
                                            # kernel tail: ship each quarter as
                                            # soon as it drains
                                            nc.sync.dma_start(
                                                out=out_d[q * 128:(q + 1) * 128,
                                                          oc * 512:(oc + 1) * 512],
                                                in_=state["ot"][:, oc * 512:(oc + 1) * 512])
                                        elif oc == 3:
                                            nc.sync.dma_start(
                                                out=out_d[q * 128:(q + 1) * 128, :],
                                                in_=state["ot"][:])
                                thunks.append(t)
                    return thunks

                pending = []   # out-proj thunks from the previous chunk
                pend_i = 0

                for qt in range(QT):
                    nkt = 4 * qt + 4  # causal: k-tiles 0..4qt+3
                    n_slots = nkt * HPC  # av-emission slots this chunk
                    slot = 0

                    def fillers():
                        # keep pending consumption proportional to progress,
                        # holding back a few thunks to cover the last head's
                        # den chain and the next chunk's out-proj warmup
                        nonlocal pend_i
                        if not pending:
                            return
                        avail = max(0, len(pending) - 8)
                        target = (avail * slot + n_slots - 1) // n_slots
                        while pend_i < min(target, avail):
                            pending[pend_i]()
                            pend_i += 1

                    # NOTE: offloading dacc adds / mask muls to Pool measured
                    # 2.4x WORSE on HW (gpsimd is a software Q7 loop, ~2x DVE
                    # cost per op, and it serialized the phase) — keep Pool to
                    # the 16 partition broadcasts only.
                    ctx_q = []  # per-head normalized ctx^T [128, 512] tiles
                    deferred_den = None
                    for h in range(HPC):
                        cps = cxp.tile([128, 512], F32, tag="cps", name=f"cps_{h}_{qt}")
                        # bf16 den accumulator on the DVE; the 128-way k sum
                        # happens once per (h, qt) in fp32 PSUM below, so the
                        # bf16 partial rounding stays ~1e-3 on the denominator.
                        # (Folding the diagonal-tile partials on the PE as
                        # ones-matmuls instead measured +21us: tiny matmuls
                        # disrupt the PE stream more than they relieve DVE.)
                        dacc = dap.tile([128, 512], BF16, tag="dacc", name=f"dacc_{h}_{qt}")
                        pts = {}
                        los = {}

                        def emit_av(kt, cps=cps, h=h, nkt=nkt, pts=pts, los=los):
                            nc.tensor.matmul(
                                cps[:, los[kt]:],
                                (vt[kt][:, h * 128:(h + 1) * 128]),
                                (pts[kt][:, los[kt]:]),
                                start=(kt == 0), stop=(kt == nkt - 1),
                            )

                        for kt in range(nkt):
                            j = kt - 4 * qt
                            # For diagonal blocks only q-cols >= 128j are
                            # unmasked; shrink the matmul N-range to skip the
                            # masked region instead of zero-filling it.
                            lo = 0 if j < 0 else j * 128
                            los[kt] = lo
                            sc = scp.tile([128, 512], F32, tag="sc", name=f"sc_{h}_{qt}_{kt}")
                            nc.tensor.matmul(
                                sc[:, lo:],
                                (kTt[h][:, kt * 128:(kt + 1) * 128]),
                                (qTt[h][:, qt * 512 + lo:(qt + 1) * 512]),
                                start=True,
                                stop=True,
                            )
                            # previous head's denominator matmul slots in here,
                            # one block after its dacc completed (no PE stall)
                            if kt == 2 and deferred_den is not None:
                                deferred_den()
                                deferred_den = None
                            pt = ptp.tile([128, 512], BF16, tag="pt", name=f"pt_{h}_{qt}_{kt}")
                            pts[kt] = pt
                            nc.scalar.activation(
                                pt[:, lo:], sc[:, lo:], EXP, scale=SCALE
                            )
                            if j >= 0:
                                # strictly-diagonal 128x128 sub-block mask
                                nc.vector.tensor_mul(
                                    pt[:, j * 128:(j + 1) * 128],
                                    pt[:, j * 128:(j + 1) * 128],
                                    tri[:],
                                )
                            with nc.allow_low_precision("bf16 den partials; final 128-way sum is fp32 in PSUM"):
                                if NONORM:
                                    pass
                                elif kt == 0:
                                    nc.vector.tensor_copy(dacc[:], pt[:])
                                else:
                                    nc.vector.tensor_add(
                                        dacc[:, lo:], dacc[:, lo:], pt[:, lo:])
                            if kt >= LAG:
                                emit_av(kt - LAG)
                                slot += 1
                                fillers()
                        for kt in range(max(0, nkt - LAG), nkt):
                            emit_av(kt)
                            slot += 1
                            fillers()

                        def make_den(h=h, qt=qt, dacc=dacc, cps=cps):
                            def den_thunk():
                                if NONORM:  # timing diag: plain PSUM drain
                                    ctx = ctxt[h][qt]
                                    nc.vector.tensor_copy(ctx[:], cps[:])
                                    ctx_q.append(ctx)
                                    return
                                den = dnp.tile([1, 512], F32, tag="den", name=f"den_{h}_{qt}")
                                nc.tensor.matmul(
                                    den[:], (ones_col[:]), (dacc[:]),
                                    start=True, stop=True,
                                )
                                recip = rcp.tile([1, 512], F32, tag="recip", name=f"recip_{h}_{qt}")
                                # ~5x faster than reciprocal(); 18-bit accuracy
                                # is plenty for the softmax denominator and the
                                # den range (>=1, <<1e38) avoids the edge cases
                                nc.vector.reciprocal_approx_fast(recip[:], den[:])
                                # reciprocal broadcast on the Pool engine frees
                                # a PSUM bank (no PE broadcast matmul)
                                rbs = rbsp.tile([128, 512], F32, tag="rbs", name=f"rbs_{h}_{qt}")
                                nc.gpsimd.partition_broadcast(rbs[:], recip[:])
                                ctx = ctxt[h][qt]
                                nc.vector.tensor_mul(ctx[:], cps[:], rbs[:])
                                ctx_q.append(ctx)
                            return den_thunk

                        deferred_den = make_den()

                    # last head's den: give the dacc adds a moment by draining
                    # a couple of fillers first
                    if pending and pend_i < len(pending):
                        pending[pend_i]()
                        pend_i += 1
                        if pend_i < len(pending):
                            pending[pend_i]()
                            pend_i += 1
                    if deferred_den is not None:
                        deferred_den()
                        deferred_den = None

                    # flush any remaining fillers, then queue this chunk's
                    # out-projection for interleaving into the next chunk
                    while pend_i < len(pending):
                        pending[pend_i]()
                        pend_i += 1
                    if STAGE >= 3:
                        pending = make_outproj_thunks(qt, ctx_q)
                        pend_i = 0
                    elif qt == QT - 1:
                        # timing diag: attention only; drain ctx tiles
                        for i, ctx in enumerate(ctx_q):
                            nc.sync.dma_start(
                                out=out_d[i * 128:(i + 1) * 128, :512],
                                in_=ctx[:])

                # last chunk's out-projection runs as a straight PE stream
                while pend_i < len(pending):
                    pending[pend_i]()
                    pend_i += 1

    nc.compile()
    return nc


def _get_built():
    global _BUILT
    if _BUILT is None:
        _BUILT = _build()
    return _BUILT


def _bf16(a):
    import ml_dtypes
    return np.ascontiguousarray(a).astype(ml_dtypes.bfloat16)


def make_in_maps(x, wq, wk, wv, wo):
    x = np.asarray(x, dtype=np.float32)
    wq = np.asarray(wq, dtype=np.float32)
    wk = np.asarray(wk, dtype=np.float32)
    wv = np.asarray(wv, dtype=np.float32)
    wo = np.asarray(wo, dtype=np.float32)
    in_maps = []
    for c in range(NCORES):
        b, hg = divmod(c, NCORES // B)
        sl = slice(hg * DC, (hg + 1) * DC)
        # pre-tile for dense-descriptor DMA (see _build dram layout notes)
        xt = x[b].T.reshape(4, 4, 128, 4, 512)          # [g, ct', p, n, c]
        xt = xt.transpose(3, 0, 2, 1, 4).reshape(QT * 4 * 128, 4 * 512)
        wqt = wq[sl, :].T.reshape(CT, 128, DC).transpose(1, 0, 2).reshape(128, CT * DC)
        wkt = wk[sl, :].T.reshape(CT, 128, DC).transpose(1, 0, 2).reshape(128, CT * DC)
        wvt = wv[sl, :].T.reshape(CT, 128, DC).transpose(1, 0, 2).reshape(128, CT * DC)
        wot = wo[:, sl].T.reshape(HPC, 128, D).transpose(1, 0, 2).reshape(128, HPC * D)
        in_maps.append({
            "xT": _bf16(xt),
            "wqT": _bf16(wqt),
            "wkT": _bf16(wkt),
            "wvT": _bf16(wvt),
            "woT": _bf16(wot),
        })
    return in_maps


def combine_outputs(results, bo):
    bo = np.asarray(bo, dtype=np.float32)
    out = np.zeros((B, S, D), dtype=np.float32)
    for c in range(NCORES):
        b = c // (NCORES // B)
        out[b] += np.asarray(results[c]["out"], dtype=np.float32)
    out += bo[None, None, :]
    return out


def kernel(x, wq, wk, wv, wo, bo):
    nc = _get_built()
    in_maps = make_in_maps(x, wq, wk, wv, wo)
    res = run_bass_kernel_spmd(nc, in_maps, core_ids=list(range(NCORES)))
    return combine_outputs(res.results, bo)


if __name__ == "__main__":
    nc = _get_built()
    print("built ok; instructions:", len(nc.inst_map))

